# revision 19
# baseline (speedup 1.0000x reference)
import numpy as np

# nn_MemEffAttention on 8 TRN2 cores.
# Core c -> (batch b = c//2, head-half hh = c%2, heads 6hh..6hh+5).
# One AllGather exchanges per-core routing partials (cosine sim vs grounding
# query); masks, group attention, scatter-combine and projection all on-device.
#
# Restructuring (exact): E = exp(S/8) once per (b,h); group masking folded
# into the value side (vtilde_g = v * member_mask, member mask appended as a
# 65th lhsT column giving the softmax denominator; kept-non-member keys
# contribute exp(0)=1 handled via +n_b on the denominator and the uniform row
# r_g). The reference's zeroed-query rows equal the uniform average r_g,
# substituted during the per-token scatter-combine (rank-1 update).

B, N, C, H, Dh = 4, 1025, 768, 12, 64
NCORES = 8
NPAD = 1152
NQ = 1026
CH = 342
NT = 9

_cache = {}


def _build(sim=False):
    import concourse.bass as bass
    import concourse.mybir as mybir
    import concourse.tile as tile
    import concourse.bacc as bacc
    import concourse.bass_isa as bass_isa
    from concourse.masks import make_identity

    f32 = mybir.dt.float32
    f32r = mybir.dt.float32r
    bf16 = mybir.dt.bfloat16
    AX = mybir.AxisListType
    OP = mybir.AluOpType
    ACT = mybir.ActivationFunctionType

    nc = bacc.Bacc("TRN2", target_bir_lowering=False, debug=False,
                   num_devices=NCORES)

    xb = nc.dram_tensor("xb", [N, C], f32, kind="ExternalInput").ap()
    wqk = nc.dram_tensor("wqk", [C, 768], f32, kind="ExternalInput").ap()
    wv = nc.dram_tensor("wv", [C, 384], f32, kind="ExternalInput").ap()
    wp = nc.dram_tensor("wp", [384, C], f32, kind="ExternalInput").ap()
    bp = nc.dram_tensor("bp", [128, 6], f32, kind="ExternalInput").ap()
    g6 = nc.dram_tensor("g6", [6, Dh], f32, kind="ExternalInput").ap()
    sel = nc.dram_tensor("sel", [4, 1], f32, kind="ExternalInput").ap()
    out_d = nc.dram_tensor("out", [C, N], f32, kind="ExternalOutput").ap()
    dbg = nc.dram_tensor("dbg", [1, 4096], f32, kind="ExternalOutput").ap()

    cc_in = nc.dram_tensor("cc_in", [1, 1024], f32).ap()
    cc_out = nc.dram_tensor("cc_out", [8, 1024], f32, addr_space="Shared").ap()
    bounce = nc.dram_tensor("bounce", [2 * NPAD], f32).ap()

    with tile.TileContext(nc) as tc:
        with (
            tc.tile_pool(name="const", bufs=1) as cp,
            tc.tile_pool(name="big", bufs=1) as bg,
            tc.tile_pool(name="xio", bufs=3) as xio,
            tc.tile_pool(name="epool", bufs=1) as ep,
            tc.tile_pool(name="rows", bufs=1) as rw,
            tc.tile_pool(name="sc1", bufs=1) as sc1,
            tc.tile_pool(name="srow", bufs=2) as srw,
            tc.tile_pool(name="wrow", bufs=2) as wr_p,
            tc.tile_pool(name="outp", bufs=2) as outp,
            tc.tile_pool(name="ps", bufs=3, space="PSUM") as ps,
            tc.tile_pool(name="pvp", bufs=2, space="PSUM") as pvp,
            tc.tile_pool(name="d2p", bufs=2, space="PSUM") as d2p,
            tc.tile_pool(name="bcp", bufs=1, space="PSUM") as bcp,
        ):
            # ---------------- constants ----------------
            ident = cp.tile([128, 128], f32)
            make_identity(nc, ident[:, :])
            ones_f = cp.tile([128, 64], f32)
            nc.vector.memset(ones_f[:, :], 1.0)
            i2f = cp.tile([128, 64], f32)
            nc.vector.memset(i2f[:, :], 0.0)
            make_identity(nc, i2f[0:64, :], nomemset=True)
            make_identity(nc, i2f[64:128, :], nomemset=True)
            I2r = cp.tile([128, 64], f32r)
            nc.vector.tensor_copy(I2r[:, :], i2f[:, :])
            sid2f = cp.tile([2, 128], f32)
            nc.vector.memset(sid2f[:, :], 0.0)
            nc.sync.dma_start(sid2f[0:1, 0:64], ones_f[0:1, 0:64])
            nc.sync.dma_start(sid2f[1:2, 64:128], ones_f[0:1, 0:64])
            sID2 = cp.tile([2, 128], f32r)
            nc.vector.tensor_copy(sID2[:, :], sid2f[:, :])
            oppair_f = cp.tile([128, 2], f32)
            nc.vector.memset(oppair_f[:, :], 0.0)
            nc.vector.memset(oppair_f[0:64, 0:1], 1.0)
            nc.vector.memset(oppair_f[64:128, 1:2], 1.0)
            oppair = cp.tile([128, 2], f32r)
            nc.vector.tensor_copy(oppair[:, :], oppair_f[:, :])
            ones2 = cp.tile([2, 1], f32r)
            nc.vector.tensor_copy(ones2[:, :], ones_f[0:2, 0:1])

            # ---------------- x -> xT (transposed, f32r) ----------------
            xT = [bg.tile([128, NPAD], f32r, tag=f"xT{k}", name=f"xTt{k}") for k in range(6)]
            for t in range(NT):
                xt = xio.tile([128, C], f32, tag="xt")
                if t == 8:
                    nc.vector.memset(xt[:, :], 0.0)
                    nc.sync.dma_start(xt[0:1, :], xb[1024:1025, :])
                else:
                    nc.sync.dma_start(xt[:, :], xb[t * 128:(t + 1) * 128, :])
                for k in range(6):
                    pt = ps.tile([128, 128], f32, tag="mm")
                    nc.tensor.transpose(pt[:, :], xt[:, k * 128:(k + 1) * 128],
                                        ident[:, :])
                    nc.vector.tensor_copy(xT[k][:, t * 128:(t + 1) * 128], pt[:, :])

            # ---------------- weights ----------------
            wqk_r = [bg.tile([128, 768], f32r, tag=f"wqk{k}", name=f"wqk_r{k}") for k in range(6)]
            wv_r = [bg.tile([128, 384], f32r, tag=f"wv{k}", name=f"wv_r{k}") for k in range(6)]
            wp_r = [bg.tile([128, 768], f32r, tag=f"wp{k}", name=f"wp_r{k}") for k in range(3)]
            for k in range(6):
                t = xio.tile([128, 768], f32, tag="xt")
                nc.sync.dma_start(t[:, :], wqk[k * 128:(k + 1) * 128, :])
                nc.vector.tensor_copy(wqk_r[k][:, :], t[:, :])
                t2 = xio.tile([128, 384], f32, tag="xt")
                nc.sync.dma_start(t2[:, :], wv[k * 128:(k + 1) * 128, :])
                nc.vector.tensor_copy(wv_r[k][:, :], t2[:, :])
            for k in range(3):
                t = xio.tile([128, 768], f32, tag="xt")
                nc.sync.dma_start(t[:, :], wp[k * 128:(k + 1) * 128, :])
                nc.vector.tensor_copy(wp_r[k][:, :], t[:, :])
            bp_t = cp.tile([128, 6], f32)
            nc.sync.dma_start(bp_t[:, :], bp[:, :])

            # ---------------- qkT (j<3: q pairs, j>=3: k pairs) -------------
            qkT = [bg.tile([128, NPAD], f32r, tag=f"qkT{j}", name=f"qkTt{j}") for j in range(6)]
            for mb in range(3):
                for ch in range(3):
                    pt = ps.tile([128, CH], f32, tag="mm")
                    for kc in range(6):
                        nc.tensor.matmul(
                            pt[:, :], wqk_r[kc][:, mb * 128:(mb + 1) * 128],
                            xT[kc][:, ch * CH:(ch + 1) * CH],
                            start=(kc == 0), stop=(kc == 5))
                    nc.scalar.copy(qkT[mb][:, ch * CH:(ch + 1) * CH], pt[:, :])

            # ---------------- routing partial ----------------
            g6t = rw.tile([6, Dh], f32, tag="g6t")
            nc.sync.dma_start(g6t[:, :], g6[:, :])
            g2 = rw.tile([6, Dh], f32, tag="g2")
            nc.vector.tensor_mul(g2[:, :], g6t[:, :], g6t[:, :])
            gss = rw.tile([6, 1], f32, tag="gss")
            nc.vector.tensor_reduce(gss[:, :], g2[:, :], axis=AX.X, op=OP.add)
            nc.scalar.sqrt(gss[:, :], gss[:, :])
            grec = rw.tile([6, 1], f32, tag="grec")
            nc.vector.reciprocal(grec[:, :], gss[:, :])
            gn = rw.tile([6, Dh], f32, tag="gn")
            nc.vector.tensor_scalar(gn[:, :], g6t[:, :], grec[:, :],
                                    1.0 / 12.0, OP.mult, OP.mult)
            gpad = rw.tile([128, 128], f32, tag="gpad")
            nc.vector.memset(gpad[:, :], 0.0)
            nc.vector.tensor_copy(gpad[0:6, 0:Dh], gn[:, :])
            gps = ps.tile([128, 128], f32, tag="mm")
            nc.tensor.transpose(gps[:, :], gpad[:, :], ident[:, :])
            gtmp = rw.tile([64, 6], f32, tag="gtmp")
            nc.scalar.copy(gtmp[:, :], gps[0:64, 0:6])
            qgn6_f = cp.tile([128, 6], f32)
            nc.vector.memset(qgn6_f[:, :], 0.0)
            nc.sync.dma_start(qgn6_f[0:64, 0:6:2], gtmp[:, 0:6:2])
            nc.sync.dma_start(qgn6_f[64:128, 1:6:2], gtmp[:, 1:6:2])
            qgn6 = cp.tile([128, 6], f32r)
            nc.vector.tensor_copy(qgn6[:, :], qgn6_f[:, :])

            partial = srw.tile([1, NPAD], f32, tag="srow")
            for ch in range(3):
                pp = ps.tile([1, CH], f32, tag="mm")
                for j in range(3):
                    sqc = sc1.tile([128, CH], f32r, tag="sqc")
                    nc.vector.tensor_mul(sqc[:, :],
                                         qkT[j][:, ch * CH:(ch + 1) * CH],
                                         qkT[j][:, ch * CH:(ch + 1) * CH])
                    p1 = ps.tile([2, CH], f32, tag="mm")
                    nc.tensor.matmul(p1[:, :], oppair[:, :], sqc[:, :],
                                     start=True, stop=True)
                    sq2c = sc1.tile([2, CH], f32, tag="sq2c")
                    nc.scalar.sqrt(sq2c[:, :], p1[:, :])
                    nc.vector.reciprocal(sq2c[:, :], sq2c[:, :])
                    p2 = ps.tile([2, CH], f32, tag="mm")
                    nc.tensor.matmul(p2[:, :], qgn6[:, 2 * j:2 * j + 2],
                                     qkT[j][:, ch * CH:(ch + 1) * CH],
                                     start=True, stop=True)
                    dot2c = sc1.tile([2, CH], f32, tag="dot2c")
                    nc.scalar.copy(dot2c[:, :], p2[:, :])
                    prod2c = sc1.tile([2, CH], f32r, tag="prod2c")
                    nc.vector.tensor_mul(prod2c[:, :], dot2c[:, :], sq2c[:, :])
                    nc.tensor.matmul(pp[:, :], ones2[:, :], prod2c[:, :],
                                     start=(j == 0), stop=(j == 2))
                nc.scalar.copy(partial[:, ch * CH:(ch + 1) * CH], pp[:, :])
            nc.sync.dma_start(cc_in[0, :], partial[0:1, 1:1025])
            if sim:
                for _c in range(NCORES):
                    nc.sync.dma_start(cc_out[_c, :], cc_in[0, :])
            else:
                nc.gpsimd.collective_compute(
                    "AllGather", OP.bypass, replica_groups=[list(range(NCORES))],
                    ins=[cc_in.opt()], outs=[cc_out.opt()])

            for mb in range(3, 6):
                for ch in range(3):
                    pt = ps.tile([128, CH], f32, tag="mm", name=f"qk2{mb}_{ch}")
                    for kc in range(6):
                        nc.tensor.matmul(
                            pt[:, :], wqk_r[kc][:, mb * 128:(mb + 1) * 128],
                            xT[kc][:, ch * CH:(ch + 1) * CH],
                            start=(kc == 0), stop=(kc == 5))
                    nc.scalar.copy(qkT[mb][:, ch * CH:(ch + 1) * CH], pt[:, :])

            # ---------------- v (normal layout, bf16) ----------------
            v_r = [bg.tile([128, 384], bf16, tag=f"v{t}", name=f"v_rt{t}") for t in range(NT)]
            for t in range(NT):
                pt = ps.tile([128, 384], f32, tag="mm")
                for kc in range(6):
                    nc.tensor.matmul(pt[:, :], xT[kc][:, t * 128:(t + 1) * 128],
                                     wv_r[kc][:, :],
                                     start=(kc == 0), stop=(kc == 5))
                nc.vector.tensor_copy(v_r[t][:, :], pt[:, :])

            # ---------------- masks (4-partition + POOL C-reduces) ----------
            qe4 = sc1.tile([4, 1024], f32, tag="qe")
            qo4 = sc1.tile([4, 1024], f32, tag="qo")
            nc.sync.dma_start(qe4[:, :], cc_out[0:8:2, :])
            nc.sync.dma_start(qo4[:, :], cc_out[1:8:2, :])
            nc.vector.tensor_add(qe4[:, :], qe4[:, :], qo4[:, :])
            for b in range(B):
                nc.sync.dma_start(dbg[0, b * 1024:(b + 1) * 1024],
                                  qe4[b:b + 1, :])
            rmin4 = rw.tile([4, 1], f32, tag="rmin4")
            rmax4 = rw.tile([4, 1], f32, tag="rmax4")
            nc.vector.tensor_reduce(rmin4[:, :], qe4[:, :], axis=AX.X, op=OP.min)
            nc.vector.tensor_reduce(rmax4[:, :], qe4[:, :], axis=AX.X, op=OP.max)
            mn = rw.tile([1, 1], f32, tag="mn")
            mx = rw.tile([1, 1], f32, tag="mx")
            nc.vector.tensor_scalar_mul(rmin4[:, :], rmin4[:, :], -1.0)
            nc.gpsimd.partition_all_reduce(rmin4[:, :], rmin4[:, :], 4,
                                           bass_isa.ReduceOp.max)
            nc.gpsimd.partition_all_reduce(rmax4[:, :], rmax4[:, :], 4,
                                           bass_isa.ReduceOp.max)
            nc.vector.tensor_copy(mn[:, :], rmin4[0:1, :])
            nc.vector.tensor_copy(mx[:, :], rmax4[0:1, :])
            tau = rw.tile([1, 1], f32, tag="tau")
            nc.vector.tensor_scalar(tau[:, :], mx[:, :], 0.9, None, OP.mult)
            nc.vector.tensor_scalar(mn[:, :], mn[:, :], -0.1, None, OP.mult)
            nc.vector.tensor_add(tau[:, :], tau[:, :], mn[:, :])
            tau4 = rw.tile([4, 1], f32, tag="tau4")
            nc.gpsimd.partition_broadcast(tau4[:, :], tau[:, :])

            selt = rw.tile([4, 1], f32, tag="selt")
            nc.sync.dma_start(selt[:, :], sel[:, :])
            pos4 = sc1.tile([4, 1024], f32, tag="qo")
            nc.vector.tensor_scalar(pos4[:, :], qe4[:, :], tau4[:, :], None,
                                    OP.is_gt)
            kp = rw.tile([1, 1024], f32, tag="kp")
            knm = rw.tile([1, 1024], f32, tag="knm")
            myp = rw.tile([1, 1024], f32, tag="myp")
            kp4 = sc1.tile([4, 1024], f32, tag="kp4")
            nc.gpsimd.partition_all_reduce(kp4[:, :], pos4[:, :], 4,
                                           bass_isa.ReduceOp.max)
            nc.vector.tensor_copy(kp[:, :], kp4[0:1, :])
            neg4 = sc1.tile([4, 1024], f32, tag="qe2")
            nc.vector.tensor_scalar(neg4[:, :], pos4[:, :], -1.0, 1.0,
                                    OP.mult, OP.add)
            nc.gpsimd.partition_all_reduce(neg4[:, :], neg4[:, :], 4,
                                           bass_isa.ReduceOp.max)
            nc.vector.tensor_copy(knm[:, :], neg4[0:1, :])
            kn = knm
            tmp4 = sc1.tile([4, 1024], f32, tag="qe")
            nc.vector.tensor_scalar_mul(tmp4[:, :], pos4[:, :], selt[:, :])
            nc.gpsimd.partition_all_reduce(tmp4[:, :], tmp4[:, :], 4,
                                           bass_isa.ReduceOp.add)
            nc.vector.tensor_copy(myp[:, :], tmp4[0:1, :])
            myn = rw.tile([1, 1024], f32, tag="myn")
            nc.vector.tensor_scalar(myn[:, :], myp[:, :], -1.0, 1.0, OP.mult, OP.add)

            # counts
            nb2 = cp.tile([2, 1], f32)
            sc = sc1.tile([1, 1024], f32, tag="tmp1k")
            nb_pos = rw.tile([1, 1], f32, tag="nb_pos")
            nc.vector.tensor_mul(sc[:, :], kp[:, :], myn[:, :])
            nc.vector.tensor_reduce(nb_pos[:, :], sc[:, :], axis=AX.X, op=OP.add)
            sc2 = sc1.tile([1, 1024], f32, tag="tmp1k")
            nb_neg = rw.tile([1, 1], f32, tag="nb_neg")
            nc.vector.tensor_mul(sc2[:, :], kn[:, :], myp[:, :])
            nc.vector.tensor_reduce(nb_neg[:, :], sc2[:, :], axis=AX.X, op=OP.add)
            nkr = rw.tile([1, 1], f32, tag="nkr")
            nc.vector.tensor_reduce(nkr[:, :], kn[:, :], axis=AX.X, op=OP.add)
            nc.vector.tensor_scalar_add(nkr[:, :], nkr[:, :], 1.0)
            nc.vector.reciprocal(nkr[:, :], nkr[:, :])
            nc.sync.dma_start(nb2[0:1, 0:1], nb_pos[0:1, :])
            nc.sync.dma_start(nb2[1:2, 0:1], nb_neg[0:1, :])

            # s rows -> partitions 0/1 of s2 (match den2 psum rows)
            s2 = cp.tile([2, NPAD], f32, tag="s2")
            s_rn_r = rw.tile([1, NPAD], f32r, tag="s_rn_r")

            r1 = srw.tile([1, NPAD], f32, tag="srow")
            nc.vector.memset(r1[:, :], 0.0)
            nc.vector.memset(r1[:, 0:1], 0.5)
            nc.vector.tensor_copy(r1[:, 1:1025], myn[:, :])
            nc.sync.dma_start(s2[1:2, :], r1[0:1, :])
            r2 = srw.tile([1, NPAD], f32, tag="srow")
            nc.vector.memset(r2[:, :], 0.0)
            nc.vector.memset(r2[:, 0:1], 0.5)
            knc = sc1.tile([1, 1024], f32, tag="tmp1k")
            nc.vector.tensor_scalar(knc[:, :], kn[:, :], -1.0, 1.0, OP.mult, OP.add)
            nc.vector.tensor_mul(r2[:, 1:1025], myp[:, :], knc[:, :])
            nc.sync.dma_start(s2[0:1, :], r2[0:1, :])
            r3 = srw.tile([1, NPAD], f32, tag="srow")
            nc.vector.memset(r3[:, :], 0.0)
            nc.vector.tensor_mul(r3[:, 1:1025], myp[:, :], kn[:, :])
            nc.vector.tensor_copy(s_rn_r[:, :], r3[:, :])
            r4 = srw.tile([1, NPAD], f32, tag="srow")
            nc.vector.memset(r4[:, :], 0.0)
            nc.vector.memset(r4[:, 0:1], 1.0)
            nc.vector.tensor_copy(r4[:, 1:1025], myp[:, :])
            nc.sync.dma_start(bounce[0:NPAD], r4[0:1, :])
            r5 = srw.tile([1, NPAD], f32, tag="srow")
            nc.vector.memset(r5[:, :], 0.0)
            nc.vector.memset(r5[:, 0:1], 1.0)
            nc.vector.tensor_scalar(r5[:, 1:1025], myp[:, :], -1.0, 1.0,
                                    OP.mult, OP.add)
            nc.sync.dma_start(bounce[NPAD:2 * NPAD], r5[0:1, :])

            a_pos_p = cp.tile([128, NT], f32)
            a_neg_p = cp.tile([128, NT], f32)
            nc.sync.dma_start(a_pos_p[:, :],
                              bounce[0:NPAD].rearrange("(t p) -> p t", p=128))
            nc.sync.dma_start(a_neg_p[:, :],
                              bounce[NPAD:2 * NPAD].rearrange("(t p) -> p t", p=128))
            a2_bf = cp.tile([128, 2 * NT], bf16)
            nc.vector.tensor_copy(a2_bf[:, 0:2 * NT:2], a_pos_p[:, :])
            nc.vector.tensor_copy(a2_bf[:, 1:2 * NT:2], a_neg_p[:, :])

            # vtilde (bf16), reusing wqk/wv slots
            vt = []
            for t in range(NT):
                tag = f"wqk{t}" if t < 6 else f"wv{t - 6}"
                vt.append(bg.tile([128, 768], bf16, tag=tag, name=f"vtt{t}"))
            for t in range(NT):
                vsrc = v_r[t][:, :].rearrange("p (h d) -> p h d", d=64)
                vdst = vt[t][:, :].rearrange("p (h x) -> p h x", x=128)
                nc.vector.tensor_scalar_mul(vdst[:, :, 0:64], vsrc,
                                            a_pos_p[:, t:t + 1])
                nc.vector.tensor_scalar_mul(vdst[:, :, 64:128], vsrc,
                                            a_neg_p[:, t:t + 1])

            # r_neg row
            prn = ps.tile([2, 384], f32, tag="mm")
            for t in range(NT):
                nc.tensor.matmul(prn[:, :], a2_bf[:, 2 * t:2 * t + 2],
                                 v_r[t][:, :],
                                 start=(t == 0), stop=(t == NT - 1))
            rr2 = rw.tile([2, 384], f32, tag="rr2")
            nc.scalar.copy(rr2[:, :], prn[:, :])
            r_row_f = rw.tile([1, 384], f32, tag="r_row_f")
            nc.sync.dma_start(r_row_f[0:1, :], rr2[1:2, :])
            r_row = rw.tile([1, 384], f32r, tag="r_row")
            nc.vector.tensor_scalar_mul(r_row[:, :], r_row_f[:, :], nkr[:, :])

            # ---------------- attention ----------------
            xoT = [bg.tile([128, NQ], f32r, tag=f"xT{j}", name=f"xoTt{j}") for j in range(3)]
            for hp in range(3):
                jq, jk = hp, 3 + hp
                xoh2 = [wr_p.tile([64, NQ], f32r, tag=f"xoh{par}", bufs=1,
                                  name=f"xoh{hp}_{par}") for par in range(2)]
                for ch in range(3):
                    Ec = {}
                    for t in range(NT):
                        for par in range(2):
                            pb = par * 64
                            sp = ps.tile([128, CH], f32, tag="mm",
                                         name=f"sp{hp}_{ch}_{t}_{par}")
                            nc.tensor.matmul(
                                sp[:, :],
                                qkT[jk][pb:pb + 64, t * 128:(t + 1) * 128],
                                qkT[jq][pb:pb + 64, ch * CH:(ch + 1) * CH],
                                start=True, stop=True, tile_position=(pb, 0))
                            e = ep.tile([128, CH], bf16, tag=f"Ec{t}_{par}",
                                        name=f"E{hp}_{ch}_{t}_{par}")
                            nc.scalar.activation(e[:, :], sp[:, :], ACT.Exp,
                                                 scale=0.125)
                            Ec[(t, par)] = e
                    for par in range(2):
                        h = 2 * hp + par
                        d2 = d2p.tile([2, CH], f32, tag="d2",
                                      name=f"d2_{hp}_{ch}_{par}")
                        for t in range(NT):
                            nc.tensor.matmul(d2[:, :], a2_bf[:, 2 * t:2 * t + 2],
                                             Ec[(t, par)][:, :],
                                             start=(t == 0), stop=(t == NT - 1))
                        w2 = wr_p.tile([2, CH], f32r, tag="w2",
                                       name=f"w2_{hp}_{ch}_{par}")
                        w2f = wr_p.tile([2, CH], f32, tag="w2f",
                                        name=f"w2f_{hp}_{ch}_{par}")
                        nc.vector.tensor_scalar_add(w2f[:, :], d2[:, :], nb2[:, :])
                        nc.vector.reciprocal(w2f[:, :], w2f[:, :])
                        nc.vector.tensor_mul(w2f[:, :], w2f[:, :],
                                             s2[:, ch * CH:(ch + 1) * CH])
                        nc.vector.tensor_copy(w2[:, :], w2f[:, :])
                        bc = bcp.tile([128, CH], f32, tag="bc",
                                      name=f"bc_{hp}_{ch}_{par}")
                        nc.tensor.matmul(bc[:, :], sID2[:, :], w2[:, :],
                                         start=True, stop=True)
                        wbc = wr_p.tile([128, CH], f32, tag="wbc",
                                        name=f"wbc_{hp}_{ch}_{par}")
                        nc.scalar.copy(wbc[:, :], bc[:, :])
                        pv = pvp.tile([128, CH], f32, tag="pv",
                                      name=f"pv_{hp}_{ch}_{par}")
                        for t in range(NT):
                            nc.tensor.matmul(pv[:, :],
                                             vt[t][:, 128 * h:128 * h + 128],
                                             Ec[(t, par)][:, :],
                                             start=(t == 0), stop=(t == NT - 1))
                        xow = wr_p.tile([128, CH], f32r, tag="xow",
                                        name=f"xow_{hp}_{ch}_{par}")
                        nc.vector.tensor_mul(xow[:, :], pv[:, :], wbc[:, :])
                        fin = pvp.tile([64, CH], f32, tag="pv",
                                       name=f"fin_{hp}_{ch}_{par}")
                        nc.tensor.matmul(fin[:, :], I2r[:, :], xow[:, :],
                                         start=True, stop=False)
                        nc.tensor.matmul(fin[:, :],
                                         r_row[0:1, 64 * h:64 * h + 64],
                                         s_rn_r[0:1, ch * CH:(ch + 1) * CH],
                                         start=False, stop=True)
                        nc.scalar.copy(xoh2[par][:, ch * CH:(ch + 1) * CH],
                                       fin[:, :])
                for par in range(2):
                    nc.sync.dma_start(xoT[hp][par * 64:(par + 1) * 64, :],
                                      xoh2[par][:, :])

            # ---------------- proj ----------------
            for mb in range(6):
                for ch in range(3):
                    pt = ps.tile([128, CH], f32, tag="mm")
                    for kc in range(3):
                        nc.tensor.matmul(pt[:, :],
                                         wp_r[kc][:, mb * 128:(mb + 1) * 128],
                                         xoT[kc][:, ch * CH:(ch + 1) * CH],
                                         start=(kc == 0), stop=(kc == 2))
                    ot = outp.tile([128, CH], f32, tag="ot")
                    nc.scalar.activation(ot[:, :], pt[:, :], ACT.Identity,
                                         bias=bp_t[:, mb:mb + 1])
                    w = min(CH, N - ch * CH)
                    nc.sync.dma_start(
                        out_d[mb * 128:(mb + 1) * 128, ch * CH:ch * CH + w],
                        ot[:, 0:w])
    nc.compile()
    return nc


def kernel(x, g_info, w_qkv, w_proj, b_proj):
    from concourse.bass_utils import run_bass_kernel_spmd

    if "nc" not in _cache:
        _cache["nc"] = _build()
    nc = _cache["nc"]

    x = np.ascontiguousarray(x, np.float32)
    g_info = np.ascontiguousarray(g_info, np.float32)
    w_qkv = np.ascontiguousarray(w_qkv, np.float32)
    w_proj = np.ascontiguousarray(w_proj, np.float32)
    b_proj = np.ascontiguousarray(b_proj, np.float32)

    g_rows = g_info[0, 0].reshape(H, Dh)
    bp_half = np.ascontiguousarray((b_proj / 2.0).reshape(6, 128).T)
    in_maps = []
    for c in range(NCORES):
        b, hh = c // 2, c % 2
        sel1 = np.zeros((4, 1), np.float32)
        sel1[b, 0] = 1.0
        wqk_c = np.concatenate(
            [w_qkv[:, hh * 384:(hh + 1) * 384],
             w_qkv[:, 768 + hh * 384:768 + (hh + 1) * 384]], axis=1)
        in_maps.append({
            "xb": x[b],
            "wqk": np.ascontiguousarray(wqk_c),
            "wv": np.ascontiguousarray(
                w_qkv[:, 1536 + hh * 384:1536 + (hh + 1) * 384]),
            "wp": np.ascontiguousarray(w_proj[hh * 384:(hh + 1) * 384, :]),
            "bp": bp_half,
            "g6": np.ascontiguousarray(g_rows[hh * 6:(hh + 1) * 6]),
            "sel": sel1,
        })
    res = run_bass_kernel_spmd(nc, in_maps, core_ids=list(range(NCORES)))
    _cache["last"] = res
    out = np.empty((B, N, C), np.float32)
    for b in range(B):
        acc = res.results[2 * b]["out"] + res.results[2 * b + 1]["out"]
        out[b] = acc.T
    return (out, g_info[1:])


# revision 21
# speedup vs baseline: 1.0269x; 1.0269x over previous
import numpy as np

# nn_MemEffAttention on 8 TRN2 cores.
# Core c -> (batch b = c//2, head-half hh = c%2, heads 6hh..6hh+5).
# One AllGather exchanges per-core routing partials (cosine sim vs grounding
# query); masks, group attention, scatter-combine and projection all on-device.
#
# Restructuring (exact): E = exp(S/8) once per (b,h); group masking folded
# into the value side (vtilde_g = v * member_mask, member mask appended as a
# 65th lhsT column giving the softmax denominator; kept-non-member keys
# contribute exp(0)=1 handled via +n_b on the denominator and the uniform row
# r_g). The reference's zeroed-query rows equal the uniform average r_g,
# substituted during the per-token scatter-combine (rank-1 update).

B, N, C, H, Dh = 4, 1025, 768, 12, 64
NCORES = 8
NPAD = 1152
NQ = 1026
CH = 342
NT = 9

_cache = {}


def _build(sim=False):
    import concourse.bass as bass
    import concourse.mybir as mybir
    import concourse.tile as tile
    import concourse.bacc as bacc
    import concourse.bass_isa as bass_isa
    from concourse.masks import make_identity

    f32 = mybir.dt.float32
    f32r = mybir.dt.float32r
    bf16 = mybir.dt.bfloat16
    AX = mybir.AxisListType
    OP = mybir.AluOpType
    ACT = mybir.ActivationFunctionType

    nc = bacc.Bacc("TRN2", target_bir_lowering=False, debug=False,
                   num_devices=NCORES)

    xb = nc.dram_tensor("xb", [N, C], f32, kind="ExternalInput").ap()
    wqk = nc.dram_tensor("wqk", [C, 768], f32, kind="ExternalInput").ap()
    wv = nc.dram_tensor("wv", [C, 384], f32, kind="ExternalInput").ap()
    wp = nc.dram_tensor("wp", [384, C], f32, kind="ExternalInput").ap()
    bp = nc.dram_tensor("bp", [128, 6], f32, kind="ExternalInput").ap()
    g6 = nc.dram_tensor("g6", [6, Dh], f32, kind="ExternalInput").ap()
    sel = nc.dram_tensor("sel", [4, 1], f32, kind="ExternalInput").ap()
    out_d = nc.dram_tensor("out", [C, N], f32, kind="ExternalOutput").ap()
    dbg = nc.dram_tensor("dbg", [1, 4096], f32, kind="ExternalOutput").ap()

    cc_in = nc.dram_tensor("cc_in", [1, 1024], f32).ap()
    cc_out = nc.dram_tensor("cc_out", [8, 1024], f32, addr_space="Shared").ap()
    bounce = nc.dram_tensor("bounce", [2 * NPAD], f32).ap()

    with tile.TileContext(nc) as tc:
        with (
            tc.tile_pool(name="const", bufs=1) as cp,
            tc.tile_pool(name="big", bufs=1) as bg,
            tc.tile_pool(name="xio", bufs=3) as xio,
            tc.tile_pool(name="epool", bufs=1) as ep,
            tc.tile_pool(name="rows", bufs=1) as rw,
            tc.tile_pool(name="sc1", bufs=1) as sc1,
            tc.tile_pool(name="srow", bufs=2) as srw,
            tc.tile_pool(name="wrow", bufs=2) as wr_p,
            tc.tile_pool(name="outp", bufs=2) as outp,
            tc.tile_pool(name="ps", bufs=3, space="PSUM") as ps,
            tc.tile_pool(name="pvp", bufs=2, space="PSUM") as pvp,
            tc.tile_pool(name="d2p", bufs=2, space="PSUM") as d2p,
            tc.tile_pool(name="bcp", bufs=1, space="PSUM") as bcp,
        ):
            # ---------------- constants ----------------
            ident = cp.tile([128, 128], f32)
            make_identity(nc, ident[:, :])
            ones_f = cp.tile([128, 64], f32)
            nc.vector.memset(ones_f[:, :], 1.0)
            i2f = cp.tile([128, 64], f32)
            nc.vector.memset(i2f[:, :], 0.0)
            make_identity(nc, i2f[0:64, :], nomemset=True)
            make_identity(nc, i2f[64:128, :], nomemset=True)
            I2r = cp.tile([128, 64], f32r)
            nc.vector.tensor_copy(I2r[:, :], i2f[:, :])
            sid2f = cp.tile([2, 128], f32)
            nc.vector.memset(sid2f[:, :], 0.0)
            nc.sync.dma_start(sid2f[0:1, 0:64], ones_f[0:1, 0:64])
            nc.sync.dma_start(sid2f[1:2, 64:128], ones_f[0:1, 0:64])
            sID2 = cp.tile([2, 128], f32r)
            nc.vector.tensor_copy(sID2[:, :], sid2f[:, :])
            oppair_f = cp.tile([128, 2], f32)
            nc.vector.memset(oppair_f[:, :], 0.0)
            nc.vector.memset(oppair_f[0:64, 0:1], 1.0)
            nc.vector.memset(oppair_f[64:128, 1:2], 1.0)
            oppair = cp.tile([128, 2], f32r)
            nc.vector.tensor_copy(oppair[:, :], oppair_f[:, :])
            ones2 = cp.tile([2, 1], f32r)
            nc.vector.tensor_copy(ones2[:, :], ones_f[0:2, 0:1])

            g6t = rw.tile([6, Dh], f32, tag="g6t")
            nc.sync.dma_start(g6t[:, :], g6[:, :])
            g2 = rw.tile([6, Dh], f32, tag="g2")
            nc.vector.tensor_mul(g2[:, :], g6t[:, :], g6t[:, :])
            gss = rw.tile([6, 1], f32, tag="gss")
            nc.vector.tensor_reduce(gss[:, :], g2[:, :], axis=AX.X, op=OP.add)
            nc.scalar.sqrt(gss[:, :], gss[:, :])
            grec = rw.tile([6, 1], f32, tag="grec")
            nc.vector.reciprocal(grec[:, :], gss[:, :])
            gn = rw.tile([6, Dh], f32, tag="gn")
            nc.vector.tensor_scalar(gn[:, :], g6t[:, :], grec[:, :],
                                    1.0 / 12.0, OP.mult, OP.mult)
            gpad = rw.tile([128, 128], f32, tag="gpad")
            nc.vector.memset(gpad[:, :], 0.0)
            nc.vector.tensor_copy(gpad[0:6, 0:Dh], gn[:, :])
            gps = ps.tile([128, 128], f32, tag="mm")
            nc.tensor.transpose(gps[:, :], gpad[:, :], ident[:, :])
            gtmp = rw.tile([64, 6], f32, tag="gtmp")
            nc.scalar.copy(gtmp[:, :], gps[0:64, 0:6])
            qgn6_f = cp.tile([128, 6], f32)
            nc.vector.memset(qgn6_f[:, :], 0.0)
            nc.sync.dma_start(qgn6_f[0:64, 0:6:2], gtmp[:, 0:6:2])
            nc.sync.dma_start(qgn6_f[64:128, 1:6:2], gtmp[:, 1:6:2])
            qgn6 = cp.tile([128, 6], f32r)
            nc.vector.tensor_copy(qgn6[:, :], qgn6_f[:, :])


            selt = rw.tile([4, 1], f32, tag="selt")
            nc.sync.dma_start(selt[:, :], sel[:, :])
            r4 = srw.tile([1, NPAD], f32, tag="srow")
            nc.vector.memset(r4[:, :], 0.0)
            nc.vector.memset(r4[:, 0:1], 1.0)

            # ---------------- x -> xT (transposed, f32r) ----------------
            xT = [bg.tile([128, NPAD], f32r, tag=f"xT{k}", name=f"xTt{k}") for k in range(6)]
            for t in range(NT):
                xt = xio.tile([128, C], f32, tag="xt")
                if t == 8:
                    nc.vector.memset(xt[:, :], 0.0)
                    nc.sync.dma_start(xt[0:1, :], xb[1024:1025, :])
                else:
                    nc.sync.dma_start(xt[:, :], xb[t * 128:(t + 1) * 128, :])
                for k in range(6):
                    pt = ps.tile([128, 128], f32, tag="mm")
                    nc.tensor.transpose(pt[:, :], xt[:, k * 128:(k + 1) * 128],
                                        ident[:, :])
                    nc.vector.tensor_copy(xT[k][:, t * 128:(t + 1) * 128], pt[:, :])

            # ---------------- weights ----------------
            wqk_r = [bg.tile([128, 768], f32r, tag=f"wqk{k}", name=f"wqk_r{k}") for k in range(6)]
            wv_r = [bg.tile([128, 384], f32r, tag=f"wv{k}", name=f"wv_r{k}") for k in range(6)]
            wp_r = [bg.tile([128, 768], f32r, tag=f"wp{k}", name=f"wp_r{k}") for k in range(3)]
            for k in range(6):
                t = xio.tile([128, 768], f32, tag="xt")
                nc.sync.dma_start(t[:, :], wqk[k * 128:(k + 1) * 128, :])
                nc.vector.tensor_copy(wqk_r[k][:, :], t[:, :])
                t2 = xio.tile([128, 384], f32, tag="xt")
                nc.sync.dma_start(t2[:, :], wv[k * 128:(k + 1) * 128, :])
                nc.vector.tensor_copy(wv_r[k][:, :], t2[:, :])
            for k in range(3):
                t = xio.tile([128, 768], f32, tag="xt")
                nc.sync.dma_start(t[:, :], wp[k * 128:(k + 1) * 128, :])
                nc.vector.tensor_copy(wp_r[k][:, :], t[:, :])
            bp_t = cp.tile([128, 6], f32)
            nc.sync.dma_start(bp_t[:, :], bp[:, :])

            # ---------------- qkT (j<3: q pairs, j>=3: k pairs) -------------
            qkT = [bg.tile([128, NPAD], f32r, tag=f"qkT{j}", name=f"qkTt{j}") for j in range(6)]
            for mb in range(3):
                for ch in range(3):
                    pt = ps.tile([128, CH], f32, tag="mm")
                    for kc in range(6):
                        nc.tensor.matmul(
                            pt[:, :], wqk_r[kc][:, mb * 128:(mb + 1) * 128],
                            xT[kc][:, ch * CH:(ch + 1) * CH],
                            start=(kc == 0), stop=(kc == 5))
                    nc.scalar.copy(qkT[mb][:, ch * CH:(ch + 1) * CH], pt[:, :])

            # ---------------- routing partial ----------------
            partial = srw.tile([1, NPAD], f32, tag="srow")
            for ch in range(3):
                pp = ps.tile([1, CH], f32, tag="mm")
                for j in range(3):
                    sqc = sc1.tile([128, CH], f32r, tag="sqc")
                    nc.vector.tensor_mul(sqc[:, :],
                                         qkT[j][:, ch * CH:(ch + 1) * CH],
                                         qkT[j][:, ch * CH:(ch + 1) * CH])
                    p1 = ps.tile([2, CH], f32, tag="mm")
                    nc.tensor.matmul(p1[:, :], oppair[:, :], sqc[:, :],
                                     start=True, stop=True)
                    sq2c = sc1.tile([2, CH], f32, tag="sq2c")
                    nc.scalar.sqrt(sq2c[:, :], p1[:, :])
                    nc.vector.reciprocal(sq2c[:, :], sq2c[:, :])
                    p2 = ps.tile([2, CH], f32, tag="mm")
                    nc.tensor.matmul(p2[:, :], qgn6[:, 2 * j:2 * j + 2],
                                     qkT[j][:, ch * CH:(ch + 1) * CH],
                                     start=True, stop=True)
                    dot2c = sc1.tile([2, CH], f32, tag="dot2c")
                    nc.scalar.copy(dot2c[:, :], p2[:, :])
                    prod2c = sc1.tile([2, CH], f32r, tag="prod2c")
                    nc.vector.tensor_mul(prod2c[:, :], dot2c[:, :], sq2c[:, :])
                    nc.tensor.matmul(pp[:, :], ones2[:, :], prod2c[:, :],
                                     start=(j == 0), stop=(j == 2))
                nc.scalar.copy(partial[:, ch * CH:(ch + 1) * CH], pp[:, :])
            nc.sync.dma_start(cc_in[0, :], partial[0:1, 1:1025])
            if sim:
                for _c in range(NCORES):
                    nc.sync.dma_start(cc_out[_c, :], cc_in[0, :])
            else:
                nc.gpsimd.collective_compute(
                    "AllGather", OP.bypass, replica_groups=[list(range(NCORES))],
                    ins=[cc_in.opt()], outs=[cc_out.opt()])

            for mb in range(3, 6):
                for ch in range(3):
                    pt = ps.tile([128, CH], f32, tag="mm", name=f"qk2{mb}_{ch}")
                    for kc in range(6):
                        nc.tensor.matmul(
                            pt[:, :], wqk_r[kc][:, mb * 128:(mb + 1) * 128],
                            xT[kc][:, ch * CH:(ch + 1) * CH],
                            start=(kc == 0), stop=(kc == 5))
                    nc.scalar.copy(qkT[mb][:, ch * CH:(ch + 1) * CH], pt[:, :])

            # ---------------- v (normal layout, bf16) ----------------
            v_r = [bg.tile([128, 384], bf16, tag=f"v{t}", name=f"v_rt{t}") for t in range(NT)]
            for t in range(NT):
                pt = ps.tile([128, 384], f32, tag="mm")
                for kc in range(6):
                    nc.tensor.matmul(pt[:, :], xT[kc][:, t * 128:(t + 1) * 128],
                                     wv_r[kc][:, :],
                                     start=(kc == 0), stop=(kc == 5))
                nc.vector.tensor_copy(v_r[t][:, :], pt[:, :])

            # ---------------- masks (4-partition + POOL C-reduces) ----------
            qe4 = sc1.tile([4, 1024], f32, tag="qe")
            qo4 = sc1.tile([4, 1024], f32, tag="qo")
            nc.sync.dma_start(qe4[:, :], cc_out[0:8:2, :])
            nc.sync.dma_start(qo4[:, :], cc_out[1:8:2, :])
            nc.vector.tensor_add(qe4[:, :], qe4[:, :], qo4[:, :])
            for b in range(B):
                nc.sync.dma_start(dbg[0, b * 1024:(b + 1) * 1024],
                                  qe4[b:b + 1, :])
            rmin4 = rw.tile([4, 1], f32, tag="rmin4")
            rmax4 = rw.tile([4, 1], f32, tag="rmax4")
            nc.vector.tensor_reduce(rmin4[:, :], qe4[:, :], axis=AX.X, op=OP.min)
            nc.vector.tensor_reduce(rmax4[:, :], qe4[:, :], axis=AX.X, op=OP.max)
            nc.vector.tensor_scalar_mul(rmin4[:, :], rmin4[:, :], -1.0)
            nc.gpsimd.partition_all_reduce(rmin4[:, :], rmin4[:, :], 4,
                                           bass_isa.ReduceOp.max)
            nc.gpsimd.partition_all_reduce(rmax4[:, :], rmax4[:, :], 4,
                                           bass_isa.ReduceOp.max)
            tau = rw.tile([1, 1], f32, tag="tau")
            nc.vector.tensor_scalar(tau[:, :], rmax4[0:1, :], 0.9, None, OP.mult)
            nc.vector.tensor_scalar(rmin4[0:1, :], rmin4[0:1, :], -0.1, None,
                                    OP.mult)
            nc.vector.tensor_add(tau[:, :], tau[:, :], rmin4[0:1, :])
            tau4 = rw.tile([4, 1], f32, tag="tau4")
            nc.gpsimd.partition_broadcast(tau4[:, :], tau[:, :])

            selt = rw.tile([4, 1], f32, tag="selt")
            nc.sync.dma_start(selt[:, :], sel[:, :])
            pos4 = sc1.tile([4, 1024], f32, tag="qo")
            nc.vector.tensor_scalar(pos4[:, :], qe4[:, :], tau4[:, :], None,
                                    OP.is_gt)
            kp = rw.tile([1, 1024], f32, tag="kp")
            knm = rw.tile([1, 1024], f32, tag="knm")
            myp = rw.tile([1, 1024], f32, tag="myp")
            kp4 = sc1.tile([4, 1024], f32, tag="kp4")
            nc.gpsimd.partition_all_reduce(kp4[:, :], pos4[:, :], 4,
                                           bass_isa.ReduceOp.max)
            nc.vector.tensor_copy(kp[:, :], kp4[0:1, :])
            neg4 = sc1.tile([4, 1024], f32, tag="qe2")
            nc.vector.tensor_scalar(neg4[:, :], pos4[:, :], -1.0, 1.0,
                                    OP.mult, OP.add)
            nc.gpsimd.partition_all_reduce(neg4[:, :], neg4[:, :], 4,
                                           bass_isa.ReduceOp.max)
            nc.vector.tensor_copy(knm[:, :], neg4[0:1, :])
            kn = knm
            tmp4 = sc1.tile([4, 1024], f32, tag="qe")
            nc.vector.tensor_scalar_mul(tmp4[:, :], pos4[:, :], selt[:, :])
            nc.gpsimd.partition_all_reduce(tmp4[:, :], tmp4[:, :], 4,
                                           bass_isa.ReduceOp.add)
            nc.vector.tensor_copy(r4[:, 1:1025], tmp4[0:1, :])
            nc.sync.dma_start(bounce[0:NPAD], r4[0:1, :])
            r5 = srw.tile([1, NPAD], f32, tag="srow")
            nc.vector.memset(r5[:, :], 0.0)
            nc.vector.memset(r5[:, 0:1], 1.0)
            nc.vector.tensor_scalar(r5[:, 1:1025], tmp4[0:1, :], -1.0, 1.0,
                                    OP.mult, OP.add)
            nc.sync.dma_start(bounce[NPAD:2 * NPAD], r5[0:1, :])
            nc.vector.tensor_copy(myp[:, :], tmp4[0:1, :])
            myn = rw.tile([1, 1024], f32, tag="myn")
            nc.vector.tensor_scalar(myn[:, :], myp[:, :], -1.0, 1.0, OP.mult, OP.add)

            # counts
            nb2 = cp.tile([2, 1], f32)
            sc = sc1.tile([1, 1024], f32, tag="tmp1k")
            nb_pos = rw.tile([1, 1], f32, tag="nb_pos")
            nc.vector.tensor_mul(sc[:, :], kp[:, :], myn[:, :])
            nc.vector.tensor_reduce(nb_pos[:, :], sc[:, :], axis=AX.X, op=OP.add)
            sc2 = sc1.tile([1, 1024], f32, tag="tmp1k")
            nb_neg = rw.tile([1, 1], f32, tag="nb_neg")
            nc.vector.tensor_mul(sc2[:, :], kn[:, :], myp[:, :])
            nc.vector.tensor_reduce(nb_neg[:, :], sc2[:, :], axis=AX.X, op=OP.add)
            nkr = rw.tile([1, 1], f32, tag="nkr")
            nc.vector.tensor_reduce(nkr[:, :], kn[:, :], axis=AX.X, op=OP.add)
            nc.vector.tensor_scalar_add(nkr[:, :], nkr[:, :], 1.0)
            nc.vector.reciprocal(nkr[:, :], nkr[:, :])
            nc.sync.dma_start(nb2[0:1, 0:1], nb_pos[0:1, :])
            nc.sync.dma_start(nb2[1:2, 0:1], nb_neg[0:1, :])

            # s rows -> partitions 0/1 of s2 (match den2 psum rows)
            s2 = cp.tile([2, NPAD], f32, tag="s2")
            s_rn_r = rw.tile([1, NPAD], f32r, tag="s_rn_r")

            r1 = srw.tile([1, NPAD], f32, tag="srow")
            nc.vector.memset(r1[:, :], 0.0)
            nc.vector.memset(r1[:, 0:1], 0.5)
            nc.vector.tensor_copy(r1[:, 1:1025], myn[:, :])
            nc.sync.dma_start(s2[1:2, :], r1[0:1, :])
            r2 = srw.tile([1, NPAD], f32, tag="srow")
            nc.vector.memset(r2[:, :], 0.0)
            nc.vector.memset(r2[:, 0:1], 0.5)
            knc = sc1.tile([1, 1024], f32, tag="tmp1k")
            nc.vector.tensor_scalar(knc[:, :], kn[:, :], -1.0, 1.0, OP.mult, OP.add)
            nc.vector.tensor_mul(r2[:, 1:1025], myp[:, :], knc[:, :])
            nc.sync.dma_start(s2[0:1, :], r2[0:1, :])
            r3 = srw.tile([1, NPAD], f32, tag="srow")
            nc.vector.memset(r3[:, :], 0.0)
            nc.vector.tensor_mul(r3[:, 1:1025], myp[:, :], kn[:, :])
            nc.vector.tensor_copy(s_rn_r[:, :], r3[:, :])

            a_pos_p = cp.tile([128, NT], f32)
            a_neg_p = cp.tile([128, NT], f32)
            nc.sync.dma_start(a_pos_p[:, :],
                              bounce[0:NPAD].rearrange("(t p) -> p t", p=128))
            nc.sync.dma_start(a_neg_p[:, :],
                              bounce[NPAD:2 * NPAD].rearrange("(t p) -> p t", p=128))
            a2_bf = cp.tile([128, 2 * NT], bf16)
            nc.vector.tensor_copy(a2_bf[:, 0:2 * NT:2], a_pos_p[:, :])
            nc.vector.tensor_copy(a2_bf[:, 1:2 * NT:2], a_neg_p[:, :])

            # vtilde (bf16), reusing wqk/wv slots
            vt = []
            for t in range(NT):
                tag = f"wqk{t}" if t < 6 else f"wv{t - 6}"
                vt.append(bg.tile([128, 768], bf16, tag=tag, name=f"vtt{t}"))
            for t in range(NT):
                vsrc = v_r[t][:, :].rearrange("p (h d) -> p h d", d=64)
                vdst = vt[t][:, :].rearrange("p (h x) -> p h x", x=128)
                nc.vector.tensor_scalar_mul(vdst[:, :, 0:64], vsrc,
                                            a_pos_p[:, t:t + 1])
                nc.vector.tensor_scalar_mul(vdst[:, :, 64:128], vsrc,
                                            a_neg_p[:, t:t + 1])

            # r_neg row
            prn = ps.tile([2, 384], f32, tag="mm")
            for t in range(NT):
                nc.tensor.matmul(prn[:, :], a2_bf[:, 2 * t:2 * t + 2],
                                 v_r[t][:, :],
                                 start=(t == 0), stop=(t == NT - 1))
            rr2 = rw.tile([2, 384], f32, tag="rr2")
            nc.scalar.copy(rr2[:, :], prn[:, :])
            r_row_f = rw.tile([1, 384], f32, tag="r_row_f")
            nc.sync.dma_start(r_row_f[0:1, :], rr2[1:2, :])
            r_row = rw.tile([1, 384], f32r, tag="r_row")
            nc.vector.tensor_scalar_mul(r_row[:, :], r_row_f[:, :], nkr[:, :])

            # ---------------- attention ----------------
            xoT = [bg.tile([128, NQ], f32r, tag=f"xT{j}", name=f"xoTt{j}") for j in range(3)]
            for hp in range(3):
                jq, jk = hp, 3 + hp
                xoh2 = [wr_p.tile([64, NQ], f32r, tag=f"xoh{par}", bufs=1,
                                  name=f"xoh{hp}_{par}") for par in range(2)]
                for ch in range(3):
                    Ec = {}
                    for t in range(NT):
                        for par in range(2):
                            pb = par * 64
                            sp = ps.tile([128, CH], f32, tag="mm",
                                         name=f"sp{hp}_{ch}_{t}_{par}")
                            nc.tensor.matmul(
                                sp[:, :],
                                qkT[jk][pb:pb + 64, t * 128:(t + 1) * 128],
                                qkT[jq][pb:pb + 64, ch * CH:(ch + 1) * CH],
                                start=True, stop=True, tile_position=(pb, 0))
                            e = ep.tile([128, CH], bf16, tag=f"Ec{t}_{par}",
                                        name=f"E{hp}_{ch}_{t}_{par}")
                            nc.scalar.activation(e[:, :], sp[:, :], ACT.Exp,
                                                 scale=0.125)
                            Ec[(t, par)] = e
                    for par in range(2):
                        h = 2 * hp + par
                        d2 = d2p.tile([2, CH], f32, tag="d2",
                                      name=f"d2_{hp}_{ch}_{par}")
                        for t in range(NT):
                            nc.tensor.matmul(d2[:, :], a2_bf[:, 2 * t:2 * t + 2],
                                             Ec[(t, par)][:, :],
                                             start=(t == 0), stop=(t == NT - 1))
                        w2 = wr_p.tile([2, CH], f32r, tag="w2",
                                       name=f"w2_{hp}_{ch}_{par}")
                        w2f = wr_p.tile([2, CH], f32, tag="w2f",
                                        name=f"w2f_{hp}_{ch}_{par}")
                        nc.vector.tensor_scalar_add(w2f[:, :], d2[:, :], nb2[:, :])
                        nc.vector.reciprocal(w2f[:, :], w2f[:, :])
                        nc.vector.tensor_mul(w2f[:, :], w2f[:, :],
                                             s2[:, ch * CH:(ch + 1) * CH])
                        nc.vector.tensor_copy(w2[:, :], w2f[:, :])
                        bc = bcp.tile([128, CH], f32, tag="bc",
                                      name=f"bc_{hp}_{ch}_{par}")
                        nc.tensor.matmul(bc[:, :], sID2[:, :], w2[:, :],
                                         start=True, stop=True)
                        wbc = wr_p.tile([128, CH], f32, tag="wbc",
                                        name=f"wbc_{hp}_{ch}_{par}")
                        nc.scalar.copy(wbc[:, :], bc[:, :])
                        pv = pvp.tile([128, CH], f32, tag="pv",
                                      name=f"pv_{hp}_{ch}_{par}")
                        for t in range(NT):
                            nc.tensor.matmul(pv[:, :],
                                             vt[t][:, 128 * h:128 * h + 128],
                                             Ec[(t, par)][:, :],
                                             start=(t == 0), stop=(t == NT - 1))
                        xow = wr_p.tile([128, CH], f32r, tag="xow",
                                        name=f"xow_{hp}_{ch}_{par}")
                        nc.vector.tensor_mul(xow[:, :], pv[:, :], wbc[:, :])
                        fin = pvp.tile([64, CH], f32, tag="pv",
                                       name=f"fin_{hp}_{ch}_{par}")
                        nc.tensor.matmul(fin[:, :], I2r[:, :], xow[:, :],
                                         start=True, stop=False)
                        nc.tensor.matmul(fin[:, :],
                                         r_row[0:1, 64 * h:64 * h + 64],
                                         s_rn_r[0:1, ch * CH:(ch + 1) * CH],
                                         start=False, stop=True)
                        nc.scalar.copy(xoh2[par][:, ch * CH:(ch + 1) * CH],
                                       fin[:, :])
                for par in range(2):
                    nc.sync.dma_start(xoT[hp][par * 64:(par + 1) * 64, :],
                                      xoh2[par][:, :])

            # ---------------- proj ----------------
            for mb in range(6):
                for ch in range(3):
                    pt = ps.tile([128, CH], f32, tag="mm")
                    for kc in range(3):
                        nc.tensor.matmul(pt[:, :],
                                         wp_r[kc][:, mb * 128:(mb + 1) * 128],
                                         xoT[kc][:, ch * CH:(ch + 1) * CH],
                                         start=(kc == 0), stop=(kc == 2))
                    ot = outp.tile([128, CH], f32, tag="ot")
                    nc.scalar.activation(ot[:, :], pt[:, :], ACT.Identity,
                                         bias=bp_t[:, mb:mb + 1])
                    w = min(CH, N - ch * CH)
                    nc.sync.dma_start(
                        out_d[mb * 128:(mb + 1) * 128, ch * CH:ch * CH + w],
                        ot[:, 0:w])
    nc.compile()
    return nc


def kernel(x, g_info, w_qkv, w_proj, b_proj):
    from concourse.bass_utils import run_bass_kernel_spmd

    if "nc" not in _cache:
        _cache["nc"] = _build()
    nc = _cache["nc"]

    x = np.ascontiguousarray(x, np.float32)
    g_info = np.ascontiguousarray(g_info, np.float32)
    w_qkv = np.ascontiguousarray(w_qkv, np.float32)
    w_proj = np.ascontiguousarray(w_proj, np.float32)
    b_proj = np.ascontiguousarray(b_proj, np.float32)

    g_rows = g_info[0, 0].reshape(H, Dh)
    bp_half = np.ascontiguousarray((b_proj / 2.0).reshape(6, 128).T)
    in_maps = []
    for c in range(NCORES):
        b, hh = c // 2, c % 2
        sel1 = np.zeros((4, 1), np.float32)
        sel1[b, 0] = 1.0
        wqk_c = np.concatenate(
            [w_qkv[:, hh * 384:(hh + 1) * 384],
             w_qkv[:, 768 + hh * 384:768 + (hh + 1) * 384]], axis=1)
        in_maps.append({
            "xb": x[b],
            "wqk": np.ascontiguousarray(wqk_c),
            "wv": np.ascontiguousarray(
                w_qkv[:, 1536 + hh * 384:1536 + (hh + 1) * 384]),
            "wp": np.ascontiguousarray(w_proj[hh * 384:(hh + 1) * 384, :]),
            "bp": bp_half,
            "g6": np.ascontiguousarray(g_rows[hh * 6:(hh + 1) * 6]),
            "sel": sel1,
        })
    res = run_bass_kernel_spmd(nc, in_maps, core_ids=list(range(NCORES)))
    _cache["last"] = res
    out = np.empty((B, N, C), np.float32)
    for b in range(B):
        acc = res.results[2 * b]["out"] + res.results[2 * b + 1]["out"]
        out[b] = acc.T
    return (out, g_info[1:])


# revision 25
# speedup vs baseline: 1.0325x; 1.0055x over previous
import numpy as np

# nn_MemEffAttention on 8 TRN2 cores.
# Core c -> (batch b = c//2, head-half hh = c%2, heads 6hh..6hh+5).
# One AllGather exchanges per-core routing partials (cosine sim vs grounding
# query); masks, group attention, scatter-combine and projection all on-device.
#
# Restructuring (exact): E = exp(S/8) once per (b,h); group masking folded
# into the value side (vtilde_g = v * member_mask, member mask appended as a
# 65th lhsT column giving the softmax denominator; kept-non-member keys
# contribute exp(0)=1 handled via +n_b on the denominator and the uniform row
# r_g). The reference's zeroed-query rows equal the uniform average r_g,
# substituted during the per-token scatter-combine (rank-1 update).

B, N, C, H, Dh = 4, 1025, 768, 12, 64
NCORES = 8
NPAD = 1152
NQ = 1026
CH = 342
NT = 9

_cache = {}


def _build(sim=False):
    import concourse.bass as bass
    import concourse.mybir as mybir
    import concourse.tile as tile
    import concourse.bacc as bacc
    import concourse.bass_isa as bass_isa
    from concourse.masks import make_identity

    f32 = mybir.dt.float32
    f32r = mybir.dt.float32r
    bf16 = mybir.dt.bfloat16
    AX = mybir.AxisListType
    OP = mybir.AluOpType
    ACT = mybir.ActivationFunctionType

    nc = bacc.Bacc("TRN2", target_bir_lowering=False, debug=False,
                   num_devices=NCORES)

    xb = nc.dram_tensor("xb", [N, C], f32, kind="ExternalInput").ap()
    wqk = nc.dram_tensor("wqk", [C, 768], f32, kind="ExternalInput").ap()
    wv = nc.dram_tensor("wv", [C, 384], f32, kind="ExternalInput").ap()
    wp = nc.dram_tensor("wp", [384, C], f32, kind="ExternalInput").ap()
    bp = nc.dram_tensor("bp", [128, 6], f32, kind="ExternalInput").ap()
    g6 = nc.dram_tensor("g6", [6, Dh], f32, kind="ExternalInput").ap()
    sel = nc.dram_tensor("sel", [4, 1], f32, kind="ExternalInput").ap()
    out_d = nc.dram_tensor("out", [C, N], f32, kind="ExternalOutput").ap()
    dbg = nc.dram_tensor("dbg", [1, 4096], f32, kind="ExternalOutput").ap()

    cc_in = nc.dram_tensor("cc_in", [1, 1024], f32).ap()
    cc_out = nc.dram_tensor("cc_out", [8, 1024], f32, addr_space="Shared").ap()
    bounce = nc.dram_tensor("bounce", [2 * NPAD], f32).ap()

    with tile.TileContext(nc) as tc:
        with (
            tc.tile_pool(name="const", bufs=1) as cp,
            tc.tile_pool(name="big", bufs=1) as bg,
            tc.tile_pool(name="xio", bufs=3) as xio,
            tc.tile_pool(name="epool", bufs=1) as ep,
            tc.tile_pool(name="rows", bufs=1) as rw,
            tc.tile_pool(name="sc1", bufs=1) as sc1,
            tc.tile_pool(name="srow", bufs=2) as srw,
            tc.tile_pool(name="wrow", bufs=2) as wr_p,
            tc.tile_pool(name="outp", bufs=2) as outp,
            tc.tile_pool(name="ps", bufs=3, space="PSUM") as ps,
            tc.tile_pool(name="pvp", bufs=2, space="PSUM") as pvp,
            tc.tile_pool(name="d2p", bufs=2, space="PSUM") as d2p,
            tc.tile_pool(name="bcp", bufs=1, space="PSUM") as bcp,
        ):
            # ---------------- constants ----------------
            ident = cp.tile([128, 128], f32)
            make_identity(nc, ident[:, :])
            ones_f = cp.tile([128, 64], f32)
            nc.vector.memset(ones_f[:, :], 1.0)
            i2f = cp.tile([128, 64], f32)
            nc.vector.memset(i2f[:, :], 0.0)
            make_identity(nc, i2f[0:64, :], nomemset=True)
            make_identity(nc, i2f[64:128, :], nomemset=True)
            I2r = cp.tile([128, 64], f32r)
            nc.vector.tensor_copy(I2r[:, :], i2f[:, :])
            sid2f = cp.tile([2, 128], f32)
            nc.vector.memset(sid2f[:, :], 0.0)
            nc.sync.dma_start(sid2f[0:1, 0:64], ones_f[0:1, 0:64])
            nc.sync.dma_start(sid2f[1:2, 64:128], ones_f[0:1, 0:64])
            sID2 = cp.tile([2, 128], f32r)
            nc.vector.tensor_copy(sID2[:, :], sid2f[:, :])
            oppair_f = cp.tile([128, 2], f32)
            nc.vector.memset(oppair_f[:, :], 0.0)
            nc.vector.memset(oppair_f[0:64, 0:1], 1.0)
            nc.vector.memset(oppair_f[64:128, 1:2], 1.0)
            oppair = cp.tile([128, 2], f32r)
            nc.vector.tensor_copy(oppair[:, :], oppair_f[:, :])
            ones2 = cp.tile([2, 1], f32r)
            nc.vector.tensor_copy(ones2[:, :], ones_f[0:2, 0:1])

            g6t = rw.tile([6, Dh], f32, tag="g6t")
            nc.sync.dma_start(g6t[:, :], g6[:, :])
            g2 = rw.tile([6, Dh], f32, tag="g2")
            nc.vector.tensor_mul(g2[:, :], g6t[:, :], g6t[:, :])
            gss = rw.tile([6, 1], f32, tag="gss")
            nc.vector.tensor_reduce(gss[:, :], g2[:, :], axis=AX.X, op=OP.add)
            nc.scalar.sqrt(gss[:, :], gss[:, :])
            grec = rw.tile([6, 1], f32, tag="grec")
            nc.vector.reciprocal(grec[:, :], gss[:, :])
            gn = rw.tile([6, Dh], f32, tag="gn")
            nc.vector.tensor_scalar(gn[:, :], g6t[:, :], grec[:, :],
                                    1.0 / 12.0, OP.mult, OP.mult)
            gpad = rw.tile([128, 128], f32, tag="gpad")
            nc.vector.memset(gpad[:, :], 0.0)
            nc.vector.tensor_copy(gpad[0:6, 0:Dh], gn[:, :])
            gps = ps.tile([128, 128], f32, tag="mm")
            nc.tensor.transpose(gps[:, :], gpad[:, :], ident[:, :])
            gtmp = rw.tile([64, 6], f32, tag="gtmp")
            nc.scalar.copy(gtmp[:, :], gps[0:64, 0:6])
            qgn6_f = cp.tile([128, 6], f32)
            nc.vector.memset(qgn6_f[:, :], 0.0)
            nc.sync.dma_start(qgn6_f[0:64, 0:6:2], gtmp[:, 0:6:2])
            nc.sync.dma_start(qgn6_f[64:128, 1:6:2], gtmp[:, 1:6:2])
            qgn6 = cp.tile([128, 6], f32r)
            nc.vector.tensor_copy(qgn6[:, :], qgn6_f[:, :])


            selt = rw.tile([4, 1], f32, tag="selt")
            nc.sync.dma_start(selt[:, :], sel[:, :])
            r4 = srw.tile([1, NPAD], f32, tag="srow")
            nc.vector.memset(r4[:, :], 0.0)
            nc.vector.memset(r4[:, 0:1], 1.0)

            # ---------------- x -> xT (transposed, f32r) ----------------
            xT = [bg.tile([128, NPAD], f32r, tag=f"xT{k}", name=f"xTt{k}") for k in range(6)]
            for t in range(NT):
                xt = xio.tile([128, C], f32, tag="xt")
                if t == 8:
                    nc.vector.memset(xt[:, :], 0.0)
                    nc.sync.dma_start(xt[0:1, :], xb[1024:1025, :])
                else:
                    nc.sync.dma_start(xt[:, :], xb[t * 128:(t + 1) * 128, :])
                for k in range(6):
                    pt = ps.tile([128, 128], f32, tag="mm")
                    nc.tensor.transpose(pt[:, :], xt[:, k * 128:(k + 1) * 128],
                                        ident[:, :])
                    nc.vector.tensor_copy(xT[k][:, t * 128:(t + 1) * 128], pt[:, :])

            # ---------------- weights ----------------
            wqk_r = [bg.tile([128, 768], f32r, tag=f"wqk{k}", name=f"wqk_r{k}") for k in range(6)]
            wv_r = [bg.tile([128, 384], f32r, tag=f"wv{k}", name=f"wv_r{k}") for k in range(6)]
            wp_r = [bg.tile([128, 768], f32r, tag=f"wp{k}", name=f"wp_r{k}") for k in range(3)]
            for k in range(6):
                t = xio.tile([128, 768], f32, tag="xt")
                nc.sync.dma_start(t[:, :], wqk[k * 128:(k + 1) * 128, :])
                nc.vector.tensor_copy(wqk_r[k][:, :], t[:, :])
                t2 = xio.tile([128, 384], f32, tag="xt")
                nc.sync.dma_start(t2[:, :], wv[k * 128:(k + 1) * 128, :])
                nc.vector.tensor_copy(wv_r[k][:, :], t2[:, :])
            for k in range(3):
                t = xio.tile([128, 768], f32, tag="xt")
                nc.sync.dma_start(t[:, :], wp[k * 128:(k + 1) * 128, :])
                nc.vector.tensor_copy(wp_r[k][:, :], t[:, :])
            bp_t = cp.tile([128, 6], f32)
            nc.sync.dma_start(bp_t[:, :], bp[:, :])

            # ---------------- qkT (j<3: q pairs, j>=3: k pairs) -------------
            qkT = [bg.tile([128, NPAD], f32r, tag=f"qkT{j}", name=f"qkTt{j}") for j in range(6)]
            for mb in range(3):
                for ch in range(3):
                    pt = ps.tile([128, CH], f32, tag="mm")
                    for kc in range(6):
                        nc.tensor.matmul(
                            pt[:, :], wqk_r[kc][:, mb * 128:(mb + 1) * 128],
                            xT[kc][:, ch * CH:(ch + 1) * CH],
                            start=(kc == 0), stop=(kc == 5))
                    nc.scalar.copy(qkT[mb][:, ch * CH:(ch + 1) * CH], pt[:, :])

            # ---------------- routing partial ----------------
            partial = srw.tile([1, NPAD], f32, tag="srow")
            for ch in range(3):
                pp = ps.tile([1, CH], f32, tag="mm")
                for j in range(3):
                    sqc = sc1.tile([128, CH], f32r, tag="sqc")
                    nc.vector.tensor_mul(sqc[:, :],
                                         qkT[j][:, ch * CH:(ch + 1) * CH],
                                         qkT[j][:, ch * CH:(ch + 1) * CH])
                    p1 = ps.tile([2, CH], f32, tag="mm")
                    nc.tensor.matmul(p1[:, :], oppair[:, :], sqc[:, :],
                                     start=True, stop=True)
                    sq2c = sc1.tile([2, CH], f32, tag="sq2c")
                    nc.scalar.sqrt(sq2c[:, :], p1[:, :])
                    nc.vector.reciprocal(sq2c[:, :], sq2c[:, :])
                    p2 = ps.tile([2, CH], f32, tag="mm")
                    nc.tensor.matmul(p2[:, :], qgn6[:, 2 * j:2 * j + 2],
                                     qkT[j][:, ch * CH:(ch + 1) * CH],
                                     start=True, stop=True)
                    dot2c = sc1.tile([2, CH], f32, tag="dot2c")
                    nc.scalar.copy(dot2c[:, :], p2[:, :])
                    prod2c = sc1.tile([2, CH], f32r, tag="prod2c")
                    nc.vector.tensor_mul(prod2c[:, :], dot2c[:, :], sq2c[:, :])
                    nc.tensor.matmul(pp[:, :], ones2[:, :], prod2c[:, :],
                                     start=(j == 0), stop=(j == 2))
                nc.scalar.copy(partial[:, ch * CH:(ch + 1) * CH], pp[:, :])
            nc.sync.dma_start(cc_in[0, :], partial[0:1, 1:1025])
            if sim:
                for _c in range(NCORES):
                    nc.sync.dma_start(cc_out[_c, :], cc_in[0, :])
            else:
                nc.gpsimd.collective_compute(
                    "AllGather", OP.bypass, replica_groups=[list(range(NCORES))],
                    ins=[cc_in.opt()], outs=[cc_out.opt()])

            for mb in range(3, 6):
                for ch in range(3):
                    pt = ps.tile([128, CH], f32, tag="mm", name=f"qk2{mb}_{ch}")
                    for kc in range(6):
                        nc.tensor.matmul(
                            pt[:, :], wqk_r[kc][:, mb * 128:(mb + 1) * 128],
                            xT[kc][:, ch * CH:(ch + 1) * CH],
                            start=(kc == 0), stop=(kc == 5))
                    nc.scalar.copy(qkT[mb][:, ch * CH:(ch + 1) * CH], pt[:, :])

            # ---------------- v (normal layout, bf16) ----------------
            v_r = [bg.tile([128, 384], bf16, tag=f"v{t}", name=f"v_rt{t}") for t in range(NT)]
            for t in range(NT):
                pt = ps.tile([128, 384], f32, tag="mm")
                for kc in range(6):
                    nc.tensor.matmul(pt[:, :], xT[kc][:, t * 128:(t + 1) * 128],
                                     wv_r[kc][:, :],
                                     start=(kc == 0), stop=(kc == 5))
                nc.vector.tensor_copy(v_r[t][:, :], pt[:, :])

            # ---------------- masks (4-partition + POOL C-reduces) ----------
            qe4 = sc1.tile([4, 1024], f32, tag="qe")
            qo4 = sc1.tile([4, 1024], f32, tag="qo")
            nc.sync.dma_start(qe4[:, :], cc_out[0:8:2, :])
            nc.sync.dma_start(qo4[:, :], cc_out[1:8:2, :])
            nc.vector.tensor_add(qe4[:, :], qe4[:, :], qo4[:, :])
            for b in range(B):
                nc.sync.dma_start(dbg[0, b * 1024:(b + 1) * 1024],
                                  qe4[b:b + 1, :])
            rmin4 = rw.tile([4, 1], f32, tag="rmin4")
            rmax4 = rw.tile([4, 1], f32, tag="rmax4")
            nc.vector.tensor_reduce(rmin4[:, :], qe4[:, :], axis=AX.X, op=OP.min)
            nc.vector.tensor_reduce(rmax4[:, :], qe4[:, :], axis=AX.X, op=OP.max)
            nc.vector.tensor_scalar_mul(rmin4[:, :], rmin4[:, :], -1.0)
            nc.gpsimd.partition_all_reduce(rmin4[:, :], rmin4[:, :], 4,
                                           bass_isa.ReduceOp.max)
            nc.gpsimd.partition_all_reduce(rmax4[:, :], rmax4[:, :], 4,
                                           bass_isa.ReduceOp.max)
            tau = rw.tile([1, 1], f32, tag="tau")
            nc.vector.tensor_scalar(tau[:, :], rmax4[0:1, :], 0.9, None, OP.mult)
            nc.vector.tensor_scalar(rmin4[0:1, :], rmin4[0:1, :], -0.1, None,
                                    OP.mult)
            nc.vector.tensor_add(tau[:, :], tau[:, :], rmin4[0:1, :])
            tau4 = rw.tile([4, 1], f32, tag="tau4")
            nc.gpsimd.partition_broadcast(tau4[:, :], tau[:, :])

            selt = rw.tile([4, 1], f32, tag="selt")
            nc.sync.dma_start(selt[:, :], sel[:, :])
            pos4 = sc1.tile([4, 1024], f32, tag="qo")
            nc.vector.tensor_scalar(pos4[:, :], qe4[:, :], tau4[:, :], None,
                                    OP.is_gt)
            kp = rw.tile([1, 1024], f32, tag="kp")
            knm = rw.tile([1, 1024], f32, tag="knm")
            myp = rw.tile([1, 1024], f32, tag="myp")
            kp4 = sc1.tile([4, 1024], f32, tag="kp4")
            nc.gpsimd.partition_all_reduce(kp4[:, :], pos4[:, :], 4,
                                           bass_isa.ReduceOp.max)
            nc.vector.tensor_copy(kp[:, :], kp4[0:1, :])
            neg4 = sc1.tile([4, 1024], f32, tag="qe2")
            nc.vector.tensor_scalar(neg4[:, :], pos4[:, :], -1.0, 1.0,
                                    OP.mult, OP.add)
            nc.gpsimd.partition_all_reduce(neg4[:, :], neg4[:, :], 4,
                                           bass_isa.ReduceOp.max)
            nc.vector.tensor_copy(knm[:, :], neg4[0:1, :])
            kn = knm
            tmp4 = sc1.tile([4, 1024], f32, tag="qe")
            nc.vector.tensor_scalar_mul(tmp4[:, :], pos4[:, :], selt[:, :])
            nc.gpsimd.partition_all_reduce(tmp4[:, :], tmp4[:, :], 4,
                                           bass_isa.ReduceOp.add)
            nc.vector.tensor_copy(r4[:, 1:1025], tmp4[0:1, :])
            nc.sync.dma_start(bounce[0:NPAD], r4[0:1, :])
            r5 = srw.tile([1, NPAD], f32, tag="srow")
            nc.vector.memset(r5[:, :], 0.0)
            nc.vector.memset(r5[:, 0:1], 1.0)
            nc.vector.tensor_scalar(r5[:, 1:1025], tmp4[0:1, :], -1.0, 1.0,
                                    OP.mult, OP.add)
            nc.sync.dma_start(bounce[NPAD:2 * NPAD], r5[0:1, :])
            nc.vector.tensor_copy(myp[:, :], tmp4[0:1, :])
            myn = rw.tile([1, 1024], f32, tag="myn")
            nc.vector.tensor_scalar(myn[:, :], myp[:, :], -1.0, 1.0, OP.mult, OP.add)

            # counts
            nb2 = cp.tile([2, 1], f32)
            sc = sc1.tile([1, 1024], f32, tag="tmp1k")
            nb_pos = rw.tile([1, 1], f32, tag="nb_pos")
            nc.vector.tensor_mul(sc[:, :], kp[:, :], myn[:, :])
            nc.vector.tensor_reduce(nb_pos[:, :], sc[:, :], axis=AX.X, op=OP.add)
            sc2 = sc1.tile([1, 1024], f32, tag="tmp1k")
            nb_neg = rw.tile([1, 1], f32, tag="nb_neg")
            nc.vector.tensor_mul(sc2[:, :], kn[:, :], myp[:, :])
            nc.vector.tensor_reduce(nb_neg[:, :], sc2[:, :], axis=AX.X, op=OP.add)
            nkr = rw.tile([1, 1], f32, tag="nkr")
            nc.vector.tensor_reduce(nkr[:, :], kn[:, :], axis=AX.X, op=OP.add)
            nc.vector.tensor_scalar_add(nkr[:, :], nkr[:, :], 1.0)
            nc.vector.reciprocal(nkr[:, :], nkr[:, :])
            nc.sync.dma_start(nb2[0:1, 0:1], nb_pos[0:1, :])
            nc.sync.dma_start(nb2[1:2, 0:1], nb_neg[0:1, :])

            # s rows -> partitions 0/1 of s2 (match den2 psum rows)
            s2 = cp.tile([2, NPAD], f32, tag="s2")
            s_rn_r = rw.tile([1, NPAD], f32r, tag="s_rn_r")

            r1 = srw.tile([1, NPAD], f32, tag="srow")
            nc.vector.memset(r1[:, :], 0.0)
            nc.vector.memset(r1[:, 0:1], 0.5)
            nc.vector.tensor_copy(r1[:, 1:1025], myn[:, :])
            nc.sync.dma_start(s2[1:2, :], r1[0:1, :])
            r2 = srw.tile([1, NPAD], f32, tag="srow")
            nc.vector.memset(r2[:, :], 0.0)
            nc.vector.memset(r2[:, 0:1], 0.5)
            knc = sc1.tile([1, 1024], f32, tag="tmp1k")
            nc.vector.tensor_scalar(knc[:, :], kn[:, :], -1.0, 1.0, OP.mult, OP.add)
            nc.vector.tensor_mul(r2[:, 1:1025], myp[:, :], knc[:, :])
            nc.sync.dma_start(s2[0:1, :], r2[0:1, :])
            r3 = srw.tile([1, NPAD], f32, tag="srow")
            nc.vector.memset(r3[:, :], 0.0)
            nc.vector.tensor_mul(r3[:, 1:1025], myp[:, :], kn[:, :])
            nc.vector.tensor_copy(s_rn_r[:, :], r3[:, :])

            a_pos_p = cp.tile([128, NT], f32)
            a_neg_p = cp.tile([128, NT], f32)
            nc.sync.dma_start(a_pos_p[:, :],
                              bounce[0:NPAD].rearrange("(t p) -> p t", p=128))
            nc.sync.dma_start(a_neg_p[:, :],
                              bounce[NPAD:2 * NPAD].rearrange("(t p) -> p t", p=128))
            a2_bf = cp.tile([128, 2 * NT], bf16)
            nc.vector.tensor_copy(a2_bf[:, 0:2 * NT:2], a_pos_p[:, :])
            nc.vector.tensor_copy(a2_bf[:, 1:2 * NT:2], a_neg_p[:, :])

            # vtilde (bf16), reusing wqk/wv slots
            vt = []
            for t in range(NT):
                tag = f"wqk{t}" if t < 6 else f"wv{t - 6}"
                vt.append(bg.tile([128, 768], bf16, tag=tag, name=f"vtt{t}"))
            for t in range(NT):
                vsrc = v_r[t][:, :].rearrange("p (h d) -> p h d", d=64)
                vdst = vt[t][:, :].rearrange("p (h x) -> p h x", x=128)
                nc.vector.tensor_scalar_mul(vdst[:, :, 0:64], vsrc,
                                            a_pos_p[:, t:t + 1])
                nc.vector.tensor_scalar_mul(vdst[:, :, 64:128], vsrc,
                                            a_neg_p[:, t:t + 1])

            # r_neg row
            prn = ps.tile([2, 384], f32, tag="mm")
            for t in range(NT):
                nc.tensor.matmul(prn[:, :], a2_bf[:, 2 * t:2 * t + 2],
                                 v_r[t][:, :],
                                 start=(t == 0), stop=(t == NT - 1))
            rr2 = rw.tile([2, 384], f32, tag="rr2")
            nc.scalar.copy(rr2[:, :], prn[:, :])
            r_row_f = rw.tile([1, 384], f32, tag="r_row_f")
            nc.sync.dma_start(r_row_f[0:1, :], rr2[1:2, :])
            r_row = rw.tile([1, 384], f32r, tag="r_row")
            nc.vector.tensor_scalar_mul(r_row[:, :], r_row_f[:, :], nkr[:, :])

            # ---------------- attention ----------------
            xoT = [bg.tile([128, NQ], f32r, tag=f"xT{j}", name=f"xoTt{j}") for j in range(3)]
            for hp in range(3):
                jq, jk = hp, 3 + hp
                xoh2 = [wr_p.tile([64, NQ], f32r, tag=f"xoh{par}", bufs=1,
                                  name=f"xoh{hp}_{par}") for par in range(2)]
                for ch in range(3):
                    Ec = {}
                    for t in range(NT):
                        for par in range(2):
                            pb = par * 64
                            sp = ps.tile([128, CH], f32, tag="mm",
                                         name=f"sp{hp}_{ch}_{t}_{par}")
                            nc.tensor.matmul(
                                sp[:, :],
                                qkT[jk][pb:pb + 64, t * 128:(t + 1) * 128],
                                qkT[jq][pb:pb + 64, ch * CH:(ch + 1) * CH],
                                start=True, stop=True, tile_position=(pb, 0))
                            e = ep.tile([128, CH], bf16, tag=f"Ec{t}_{par}",
                                        name=f"E{hp}_{ch}_{t}_{par}")
                            nc.scalar.activation(e[:, :], sp[:, :], ACT.Exp,
                                                 scale=0.125)
                            Ec[(t, par)] = e
                    for par in range(2):
                        h = 2 * hp + par
                        d2 = d2p.tile([2, CH], f32, tag="d2",
                                      name=f"d2_{hp}_{ch}_{par}")
                        for t in range(NT):
                            nc.tensor.matmul(d2[:, :], a2_bf[:, 2 * t:2 * t + 2],
                                             Ec[(t, par)][:, :],
                                             start=(t == 0), stop=(t == NT - 1))
                        w2 = wr_p.tile([2, CH], f32r, tag="w2",
                                       name=f"w2_{hp}_{ch}_{par}")
                        w2f = wr_p.tile([2, CH], f32, tag="w2f",
                                        name=f"w2f_{hp}_{ch}_{par}")
                        nc.vector.tensor_scalar_add(w2f[:, :], d2[:, :], nb2[:, :])
                        nc.vector.reciprocal(w2f[:, :], w2f[:, :])
                        nc.vector.tensor_mul(w2f[:, :], w2f[:, :],
                                             s2[:, ch * CH:(ch + 1) * CH])
                        nc.vector.tensor_copy(w2[:, :], w2f[:, :])
                        bc = bcp.tile([128, CH], f32, tag="bc",
                                      name=f"bc_{hp}_{ch}_{par}")
                        nc.tensor.matmul(bc[:, :], sID2[:, :], w2[:, :],
                                         start=True, stop=True)
                        wbc = wr_p.tile([128, CH], f32, tag="wbc",
                                        name=f"wbc_{hp}_{ch}_{par}")
                        nc.scalar.copy(wbc[:, :], bc[:, :])
                        pv = pvp.tile([128, CH], f32, tag="pv",
                                      name=f"pv_{hp}_{ch}_{par}")
                        for t in range(NT):
                            nc.tensor.matmul(pv[:, :],
                                             vt[t][:, 128 * h:128 * h + 128],
                                             Ec[(t, par)][:, :],
                                             start=(t == 0), stop=(t == NT - 1))
                        xow = wr_p.tile([128, CH], f32r, tag="xow",
                                        name=f"xow_{hp}_{ch}_{par}")
                        nc.vector.tensor_mul(xow[:, :], pv[:, :], wbc[:, :])
                        fin = pvp.tile([64, CH], f32, tag="pv",
                                       name=f"fin_{hp}_{ch}_{par}")
                        nc.tensor.matmul(fin[:, :], I2r[:, :], xow[:, :],
                                         start=True, stop=False)
                        nc.tensor.matmul(fin[:, :],
                                         r_row[0:1, 64 * h:64 * h + 64],
                                         s_rn_r[0:1, ch * CH:(ch + 1) * CH],
                                         start=False, stop=True)
                        nc.scalar.copy(xoh2[par][:, ch * CH:(ch + 1) * CH],
                                       fin[:, :])
                for par in range(2):
                    nc.sync.dma_start(xoT[hp][par * 64:(par + 1) * 64, :],
                                      xoh2[par][:, :])

            # ---------------- proj ----------------
            for mb in range(6):
                for ch in range(3):
                    pt = ps.tile([128, CH], f32, tag="mm")
                    for kc in range(3):
                        nc.tensor.matmul(pt[:, :],
                                         wp_r[kc][:, mb * 128:(mb + 1) * 128],
                                         xoT[kc][:, ch * CH:(ch + 1) * CH],
                                         start=(kc == 0), stop=(kc == 2))
                    ot = outp.tile([128, CH], f32, tag="ot")
                    nc.scalar.activation(ot[:, :], pt[:, :], ACT.Identity,
                                         bias=bp_t[:, mb:mb + 1])
                    w = min(CH, N - ch * CH)
                    nc.sync.dma_start(
                        out_d[mb * 128:(mb + 1) * 128, ch * CH:ch * CH + w],
                        ot[:, 0:w])
    nc.compile()
    return nc


def kernel(x, g_info, w_qkv, w_proj, b_proj):
    from concourse.bass_utils import run_bass_kernel_spmd

    if "nc" not in _cache:
        _cache["nc"] = _build()
    nc = _cache["nc"]

    x = np.ascontiguousarray(x, np.float32)
    g_info = np.ascontiguousarray(g_info, np.float32)
    w_qkv = np.ascontiguousarray(w_qkv, np.float32)
    w_proj = np.ascontiguousarray(w_proj, np.float32)
    b_proj = np.ascontiguousarray(b_proj, np.float32)

    g_rows = g_info[0, 0].reshape(H, Dh)
    bp_half = np.ascontiguousarray((b_proj / 2.0).reshape(6, 128).T)
    in_maps = []
    for c in range(NCORES):
        b, hh = c // 2, c % 2
        sel1 = np.zeros((4, 1), np.float32)
        sel1[b, 0] = 1.0
        wqk_c = np.concatenate(
            [w_qkv[:, hh * 384:(hh + 1) * 384],
             w_qkv[:, 768 + hh * 384:768 + (hh + 1) * 384]], axis=1)
        in_maps.append({
            "xb": x[b],
            "wqk": np.ascontiguousarray(wqk_c),
            "wv": np.ascontiguousarray(
                w_qkv[:, 1536 + hh * 384:1536 + (hh + 1) * 384]),
            "wp": np.ascontiguousarray(w_proj[hh * 384:(hh + 1) * 384, :]),
            "bp": bp_half,
            "g6": np.ascontiguousarray(g_rows[hh * 6:(hh + 1) * 6]),
            "sel": sel1,
        })
    res = run_bass_kernel_spmd(nc, in_maps, core_ids=list(range(NCORES)))
    _cache["last"] = res
    out = np.empty((B, N, C), np.float32)
    for b in range(B):
        acc = res.results[2 * b]["out"] + res.results[2 * b + 1]["out"]
        out[b] = acc.T
    return (out, g_info[1:])


# revision 26
# speedup vs baseline: 1.0680x; 1.0344x over previous
import numpy as np

# nn_MemEffAttention on 8 TRN2 cores.
# Core c -> (batch b = c//2, head-half hh = c%2, heads 6hh..6hh+5).
# One AllGather exchanges per-core routing partials (cosine sim vs grounding
# query); masks, group attention, scatter-combine and projection all on-device.
#
# Restructuring (exact): E = exp(S/8) once per (b,h); group masking folded
# into the value side (vtilde_g = v * member_mask, member mask appended as a
# 65th lhsT column giving the softmax denominator; kept-non-member keys
# contribute exp(0)=1 handled via +n_b on the denominator and the uniform row
# r_g). The reference's zeroed-query rows equal the uniform average r_g,
# substituted during the per-token scatter-combine (rank-1 update).

B, N, C, H, Dh = 4, 1025, 768, 12, 64
NCORES = 8
NPAD = 1152
NQ = 1026
CH = 342
NT = 9

_cache = {}


def _build(sim=False):
    import concourse.bass as bass
    import concourse.mybir as mybir
    import concourse.tile as tile
    import concourse.bacc as bacc
    import concourse.bass_isa as bass_isa
    from concourse.masks import make_identity

    f32 = mybir.dt.float32
    f32r = mybir.dt.float32r
    bf16 = mybir.dt.bfloat16
    AX = mybir.AxisListType
    OP = mybir.AluOpType
    ACT = mybir.ActivationFunctionType

    nc = bacc.Bacc("TRN2", target_bir_lowering=False, debug=False,
                   num_devices=NCORES)

    xb = nc.dram_tensor("xb", [N, C], f32, kind="ExternalInput").ap()
    wqk = nc.dram_tensor("wqk", [C, 768], f32, kind="ExternalInput").ap()
    wv = nc.dram_tensor("wv", [C, 384], f32, kind="ExternalInput").ap()
    wp = nc.dram_tensor("wp", [384, C], f32, kind="ExternalInput").ap()
    bp = nc.dram_tensor("bp", [128, 6], f32, kind="ExternalInput").ap()
    g6 = nc.dram_tensor("g6", [6, Dh], f32, kind="ExternalInput").ap()
    sel = nc.dram_tensor("sel", [4, 1], f32, kind="ExternalInput").ap()
    out_d = nc.dram_tensor("out", [C, N], f32, kind="ExternalOutput").ap()
    dbg = nc.dram_tensor("dbg", [1, 4096], f32, kind="ExternalOutput").ap()

    cc_in = nc.dram_tensor("cc_in", [1, 1024], f32).ap()
    cc_out = nc.dram_tensor("cc_out", [8, 1024], f32, addr_space="Shared").ap()
    bounce = nc.dram_tensor("bounce", [2 * NPAD], f32).ap()

    with tile.TileContext(nc) as tc:
        with (
            tc.tile_pool(name="const", bufs=1) as cp,
            tc.tile_pool(name="big", bufs=1) as bg,
            tc.tile_pool(name="xio", bufs=3) as xio,
            tc.tile_pool(name="epool", bufs=1) as ep,
            tc.tile_pool(name="rows", bufs=1) as rw,
            tc.tile_pool(name="sc1", bufs=1) as sc1,
            tc.tile_pool(name="srow", bufs=2) as srw,
            tc.tile_pool(name="wrow", bufs=2) as wr_p,
            tc.tile_pool(name="outp", bufs=2) as outp,
            tc.tile_pool(name="ps", bufs=3, space="PSUM") as ps,
            tc.tile_pool(name="pvp", bufs=3, space="PSUM") as pvp,
            tc.tile_pool(name="d2p", bufs=1, space="PSUM") as d2p,
            tc.tile_pool(name="bcp", bufs=1, space="PSUM") as bcp,
        ):
            # ---------------- constants ----------------
            ident = cp.tile([128, 128], f32)
            make_identity(nc, ident[:, :])
            ones_f = cp.tile([128, 64], f32)
            nc.vector.memset(ones_f[:, :], 1.0)
            i2f = cp.tile([128, 64], f32)
            nc.vector.memset(i2f[:, :], 0.0)
            make_identity(nc, i2f[0:64, :], nomemset=True)
            make_identity(nc, i2f[64:128, :], nomemset=True)
            I2r = cp.tile([128, 64], f32r)
            nc.vector.tensor_copy(I2r[:, :], i2f[:, :])
            sid2f = cp.tile([2, 128], f32)
            nc.vector.memset(sid2f[:, :], 0.0)
            nc.sync.dma_start(sid2f[0:1, 0:64], ones_f[0:1, 0:64])
            nc.sync.dma_start(sid2f[1:2, 64:128], ones_f[0:1, 0:64])
            sID2 = cp.tile([2, 128], f32r)
            nc.vector.tensor_copy(sID2[:, :], sid2f[:, :])
            oppair_f = cp.tile([128, 2], f32)
            nc.vector.memset(oppair_f[:, :], 0.0)
            nc.vector.memset(oppair_f[0:64, 0:1], 1.0)
            nc.vector.memset(oppair_f[64:128, 1:2], 1.0)
            oppair = cp.tile([128, 2], f32r)
            nc.vector.tensor_copy(oppair[:, :], oppair_f[:, :])
            ones2 = cp.tile([2, 1], f32r)
            nc.vector.tensor_copy(ones2[:, :], ones_f[0:2, 0:1])

            g6t = rw.tile([6, Dh], f32, tag="g6t")
            nc.sync.dma_start(g6t[:, :], g6[:, :])
            g2 = rw.tile([6, Dh], f32, tag="g2")
            nc.vector.tensor_mul(g2[:, :], g6t[:, :], g6t[:, :])
            gss = rw.tile([6, 1], f32, tag="gss")
            nc.vector.tensor_reduce(gss[:, :], g2[:, :], axis=AX.X, op=OP.add)
            nc.scalar.sqrt(gss[:, :], gss[:, :])
            grec = rw.tile([6, 1], f32, tag="grec")
            nc.vector.reciprocal(grec[:, :], gss[:, :])
            gn = rw.tile([6, Dh], f32, tag="gn")
            nc.vector.tensor_scalar(gn[:, :], g6t[:, :], grec[:, :],
                                    1.0 / 12.0, OP.mult, OP.mult)
            gpad = rw.tile([128, 128], f32, tag="gpad")
            nc.vector.memset(gpad[:, :], 0.0)
            nc.vector.tensor_copy(gpad[0:6, 0:Dh], gn[:, :])
            gps = ps.tile([128, 128], f32, tag="mm")
            nc.tensor.transpose(gps[:, :], gpad[:, :], ident[:, :])
            gtmp = rw.tile([64, 6], f32, tag="gtmp")
            nc.scalar.copy(gtmp[:, :], gps[0:64, 0:6])
            qgn6_f = cp.tile([128, 6], f32)
            nc.vector.memset(qgn6_f[:, :], 0.0)
            nc.sync.dma_start(qgn6_f[0:64, 0:6:2], gtmp[:, 0:6:2])
            nc.sync.dma_start(qgn6_f[64:128, 1:6:2], gtmp[:, 1:6:2])
            qgn6 = cp.tile([128, 6], f32r)
            nc.vector.tensor_copy(qgn6[:, :], qgn6_f[:, :])


            selt = rw.tile([4, 1], f32, tag="selt")
            nc.sync.dma_start(selt[:, :], sel[:, :])
            r4 = srw.tile([1, NPAD], f32, tag="srow")
            nc.vector.memset(r4[:, :], 0.0)
            nc.vector.memset(r4[:, 0:1], 1.0)

            # ---------------- x -> xT (transposed, f32r) ----------------
            xT = [bg.tile([128, NPAD], f32r, tag=f"xT{k}", name=f"xTt{k}") for k in range(6)]
            for t in range(NT):
                xt = xio.tile([128, C], f32, tag="xt")
                if t == 8:
                    nc.vector.memset(xt[:, :], 0.0)
                    nc.sync.dma_start(xt[0:1, :], xb[1024:1025, :])
                else:
                    nc.sync.dma_start(xt[:, :], xb[t * 128:(t + 1) * 128, :])
                for k in range(6):
                    pt = ps.tile([128, 128], f32, tag="mm")
                    nc.tensor.transpose(pt[:, :], xt[:, k * 128:(k + 1) * 128],
                                        ident[:, :])
                    nc.vector.tensor_copy(xT[k][:, t * 128:(t + 1) * 128], pt[:, :])

            # ---------------- weights ----------------
            wqk_r = [bg.tile([128, 768], f32r, tag=f"wqk{k}", name=f"wqk_r{k}") for k in range(6)]
            wv_r = [bg.tile([128, 384], f32r, tag=f"wv{k}", name=f"wv_r{k}") for k in range(6)]
            wp_r = [bg.tile([128, 768], f32r, tag=f"wp{k}", name=f"wp_r{k}") for k in range(3)]
            for k in range(6):
                t = xio.tile([128, 768], f32, tag="xt")
                nc.sync.dma_start(t[:, :], wqk[k * 128:(k + 1) * 128, :])
                nc.vector.tensor_copy(wqk_r[k][:, :], t[:, :])
                t2 = xio.tile([128, 384], f32, tag="xt")
                nc.sync.dma_start(t2[:, :], wv[k * 128:(k + 1) * 128, :])
                nc.vector.tensor_copy(wv_r[k][:, :], t2[:, :])
            for k in range(3):
                t = xio.tile([128, 768], f32, tag="xt")
                nc.sync.dma_start(t[:, :], wp[k * 128:(k + 1) * 128, :])
                nc.vector.tensor_copy(wp_r[k][:, :], t[:, :])
            bp_t = cp.tile([128, 6], f32)
            nc.sync.dma_start(bp_t[:, :], bp[:, :])

            # ---------------- qkT (j<3: q pairs, j>=3: k pairs) -------------
            qkT = [bg.tile([128, NPAD], f32r, tag=f"qkT{j}", name=f"qkTt{j}") for j in range(6)]
            for mb in range(3):
                for ch in range(3):
                    pt = ps.tile([128, CH], f32, tag="mm")
                    for kc in range(6):
                        nc.tensor.matmul(
                            pt[:, :], wqk_r[kc][:, mb * 128:(mb + 1) * 128],
                            xT[kc][:, ch * CH:(ch + 1) * CH],
                            start=(kc == 0), stop=(kc == 5))
                    nc.scalar.copy(qkT[mb][:, ch * CH:(ch + 1) * CH], pt[:, :])

            # ---------------- routing partial ----------------
            partial = srw.tile([1, NPAD], f32, tag="srow")
            for ch in range(3):
                pp = ps.tile([1, CH], f32, tag="mm")
                for j in range(3):
                    sqc = sc1.tile([128, CH], f32r, tag="sqc")
                    nc.vector.tensor_mul(sqc[:, :],
                                         qkT[j][:, ch * CH:(ch + 1) * CH],
                                         qkT[j][:, ch * CH:(ch + 1) * CH])
                    p1 = ps.tile([2, CH], f32, tag="mm")
                    nc.tensor.matmul(p1[:, :], oppair[:, :], sqc[:, :],
                                     start=True, stop=True)
                    sq2c = sc1.tile([2, CH], f32, tag="sq2c")
                    nc.scalar.sqrt(sq2c[:, :], p1[:, :])
                    nc.vector.reciprocal(sq2c[:, :], sq2c[:, :])
                    p2 = ps.tile([2, CH], f32, tag="mm")
                    nc.tensor.matmul(p2[:, :], qgn6[:, 2 * j:2 * j + 2],
                                     qkT[j][:, ch * CH:(ch + 1) * CH],
                                     start=True, stop=True)
                    dot2c = sc1.tile([2, CH], f32, tag="dot2c")
                    nc.scalar.copy(dot2c[:, :], p2[:, :])
                    prod2c = sc1.tile([2, CH], f32r, tag="prod2c")
                    nc.vector.tensor_mul(prod2c[:, :], dot2c[:, :], sq2c[:, :])
                    nc.tensor.matmul(pp[:, :], ones2[:, :], prod2c[:, :],
                                     start=(j == 0), stop=(j == 2))
                nc.scalar.copy(partial[:, ch * CH:(ch + 1) * CH], pp[:, :])
            nc.sync.dma_start(cc_in[0, :], partial[0:1, 1:1025])
            if sim:
                for _c in range(NCORES):
                    nc.sync.dma_start(cc_out[_c, :], cc_in[0, :])
            else:
                nc.gpsimd.collective_compute(
                    "AllGather", OP.bypass, replica_groups=[list(range(NCORES))],
                    ins=[cc_in.opt()], outs=[cc_out.opt()])

            for mb in range(3, 6):
                for ch in range(3):
                    pt = ps.tile([128, CH], f32, tag="mm", name=f"qk2{mb}_{ch}")
                    for kc in range(6):
                        nc.tensor.matmul(
                            pt[:, :], wqk_r[kc][:, mb * 128:(mb + 1) * 128],
                            xT[kc][:, ch * CH:(ch + 1) * CH],
                            start=(kc == 0), stop=(kc == 5))
                    nc.scalar.copy(qkT[mb][:, ch * CH:(ch + 1) * CH], pt[:, :])

            # ---------------- v (normal layout, bf16) ----------------
            v_r = [bg.tile([128, 384], bf16, tag=f"v{t}", name=f"v_rt{t}") for t in range(NT)]
            for t in range(NT):
                pt = ps.tile([128, 384], f32, tag="mm")
                for kc in range(6):
                    nc.tensor.matmul(pt[:, :], xT[kc][:, t * 128:(t + 1) * 128],
                                     wv_r[kc][:, :],
                                     start=(kc == 0), stop=(kc == 5))
                nc.vector.tensor_copy(v_r[t][:, :], pt[:, :])

            # ---------------- masks (4-partition + POOL C-reduces) ----------
            qe4 = sc1.tile([4, 1024], f32, tag="qe")
            qo4 = sc1.tile([4, 1024], f32, tag="qo")
            nc.sync.dma_start(qe4[:, :], cc_out[0:8:2, :])
            nc.sync.dma_start(qo4[:, :], cc_out[1:8:2, :])
            nc.vector.tensor_add(qe4[:, :], qe4[:, :], qo4[:, :])
            for b in range(B):
                nc.sync.dma_start(dbg[0, b * 1024:(b + 1) * 1024],
                                  qe4[b:b + 1, :])
            rmin4 = rw.tile([4, 1], f32, tag="rmin4")
            rmax4 = rw.tile([4, 1], f32, tag="rmax4")
            nc.vector.tensor_reduce(rmin4[:, :], qe4[:, :], axis=AX.X, op=OP.min)
            nc.vector.tensor_reduce(rmax4[:, :], qe4[:, :], axis=AX.X, op=OP.max)
            nc.vector.tensor_scalar_mul(rmin4[:, :], rmin4[:, :], -1.0)
            nc.gpsimd.partition_all_reduce(rmin4[:, :], rmin4[:, :], 4,
                                           bass_isa.ReduceOp.max)
            nc.gpsimd.partition_all_reduce(rmax4[:, :], rmax4[:, :], 4,
                                           bass_isa.ReduceOp.max)
            tau = rw.tile([1, 1], f32, tag="tau")
            nc.vector.tensor_scalar(tau[:, :], rmax4[0:1, :], 0.9, None, OP.mult)
            nc.vector.tensor_scalar(rmin4[0:1, :], rmin4[0:1, :], -0.1, None,
                                    OP.mult)
            nc.vector.tensor_add(tau[:, :], tau[:, :], rmin4[0:1, :])
            tau4 = rw.tile([4, 1], f32, tag="tau4")
            nc.gpsimd.partition_broadcast(tau4[:, :], tau[:, :])

            selt = rw.tile([4, 1], f32, tag="selt")
            nc.sync.dma_start(selt[:, :], sel[:, :])
            pos4 = sc1.tile([4, 1024], f32, tag="qo")
            nc.vector.tensor_scalar(pos4[:, :], qe4[:, :], tau4[:, :], None,
                                    OP.is_gt)
            kp = rw.tile([1, 1024], f32, tag="kp")
            knm = rw.tile([1, 1024], f32, tag="knm")
            myp = rw.tile([1, 1024], f32, tag="myp")
            kp4 = sc1.tile([4, 1024], f32, tag="kp4")
            nc.gpsimd.partition_all_reduce(kp4[:, :], pos4[:, :], 4,
                                           bass_isa.ReduceOp.max)
            nc.vector.tensor_copy(kp[:, :], kp4[0:1, :])
            neg4 = sc1.tile([4, 1024], f32, tag="qe2")
            nc.vector.tensor_scalar(neg4[:, :], pos4[:, :], -1.0, 1.0,
                                    OP.mult, OP.add)
            nc.gpsimd.partition_all_reduce(neg4[:, :], neg4[:, :], 4,
                                           bass_isa.ReduceOp.max)
            nc.vector.tensor_copy(knm[:, :], neg4[0:1, :])
            kn = knm
            tmp4 = sc1.tile([4, 1024], f32, tag="qe")
            nc.vector.tensor_scalar_mul(tmp4[:, :], pos4[:, :], selt[:, :])
            nc.gpsimd.partition_all_reduce(tmp4[:, :], tmp4[:, :], 4,
                                           bass_isa.ReduceOp.add)
            nc.vector.tensor_copy(r4[:, 1:1025], tmp4[0:1, :])
            nc.sync.dma_start(bounce[0:NPAD], r4[0:1, :])
            r5 = srw.tile([1, NPAD], f32, tag="srow")
            nc.vector.memset(r5[:, :], 0.0)
            nc.vector.memset(r5[:, 0:1], 1.0)
            nc.vector.tensor_scalar(r5[:, 1:1025], tmp4[0:1, :], -1.0, 1.0,
                                    OP.mult, OP.add)
            nc.sync.dma_start(bounce[NPAD:2 * NPAD], r5[0:1, :])
            nc.vector.tensor_copy(myp[:, :], tmp4[0:1, :])
            myn = rw.tile([1, 1024], f32, tag="myn")
            nc.vector.tensor_scalar(myn[:, :], myp[:, :], -1.0, 1.0, OP.mult, OP.add)

            # counts
            nb2 = cp.tile([2, 1], f32)
            sc = sc1.tile([1, 1024], f32, tag="tmp1k")
            nb_pos = rw.tile([1, 1], f32, tag="nb_pos")
            nc.vector.tensor_mul(sc[:, :], kp[:, :], myn[:, :])
            nc.vector.tensor_reduce(nb_pos[:, :], sc[:, :], axis=AX.X, op=OP.add)
            sc2 = sc1.tile([1, 1024], f32, tag="tmp1k")
            nb_neg = rw.tile([1, 1], f32, tag="nb_neg")
            nc.vector.tensor_mul(sc2[:, :], kn[:, :], myp[:, :])
            nc.vector.tensor_reduce(nb_neg[:, :], sc2[:, :], axis=AX.X, op=OP.add)
            nkr = rw.tile([1, 1], f32, tag="nkr")
            nc.vector.tensor_reduce(nkr[:, :], kn[:, :], axis=AX.X, op=OP.add)
            nc.vector.tensor_scalar_add(nkr[:, :], nkr[:, :], 1.0)
            nc.vector.reciprocal(nkr[:, :], nkr[:, :])
            nc.sync.dma_start(nb2[0:1, 0:1], nb_pos[0:1, :])
            nc.sync.dma_start(nb2[1:2, 0:1], nb_neg[0:1, :])

            # s rows -> partitions 0/1 of s2 (match den2 psum rows)
            s2 = cp.tile([2, NPAD], f32, tag="s2")
            s_rn_r = rw.tile([1, NPAD], f32r, tag="s_rn_r")

            r1 = srw.tile([1, NPAD], f32, tag="srow")
            nc.vector.memset(r1[:, :], 0.0)
            nc.vector.memset(r1[:, 0:1], 0.5)
            nc.vector.tensor_copy(r1[:, 1:1025], myn[:, :])
            nc.sync.dma_start(s2[1:2, :], r1[0:1, :])
            r2 = srw.tile([1, NPAD], f32, tag="srow")
            nc.vector.memset(r2[:, :], 0.0)
            nc.vector.memset(r2[:, 0:1], 0.5)
            knc = sc1.tile([1, 1024], f32, tag="tmp1k")
            nc.vector.tensor_scalar(knc[:, :], kn[:, :], -1.0, 1.0, OP.mult, OP.add)
            nc.vector.tensor_mul(r2[:, 1:1025], myp[:, :], knc[:, :])
            nc.sync.dma_start(s2[0:1, :], r2[0:1, :])
            r3 = srw.tile([1, NPAD], f32, tag="srow")
            nc.vector.memset(r3[:, :], 0.0)
            nc.vector.tensor_mul(r3[:, 1:1025], myp[:, :], kn[:, :])
            nc.vector.tensor_copy(s_rn_r[:, :], r3[:, :])

            a_pos_p = cp.tile([128, NT], f32)
            a_neg_p = cp.tile([128, NT], f32)
            nc.sync.dma_start(a_pos_p[:, :],
                              bounce[0:NPAD].rearrange("(t p) -> p t", p=128))
            nc.sync.dma_start(a_neg_p[:, :],
                              bounce[NPAD:2 * NPAD].rearrange("(t p) -> p t", p=128))
            a2_bf = cp.tile([128, 2 * NT], bf16)
            nc.vector.tensor_copy(a2_bf[:, 0:2 * NT:2], a_pos_p[:, :])
            nc.vector.tensor_copy(a2_bf[:, 1:2 * NT:2], a_neg_p[:, :])

            # vtilde (bf16), reusing wqk/wv slots
            vt = []
            for t in range(NT):
                tag = f"wqk{t}" if t < 6 else f"wv{t - 6}"
                vt.append(bg.tile([128, 768], bf16, tag=tag, name=f"vtt{t}"))
            for t in range(NT):
                vsrc = v_r[t][:, :].rearrange("p (h d) -> p h d", d=64)
                vdst = vt[t][:, :].rearrange("p (h x) -> p h x", x=128)
                nc.vector.tensor_scalar_mul(vdst[:, :, 0:64], vsrc,
                                            a_pos_p[:, t:t + 1])
                nc.vector.tensor_scalar_mul(vdst[:, :, 64:128], vsrc,
                                            a_neg_p[:, t:t + 1])

            # r_neg row
            prn = ps.tile([2, 384], f32, tag="mm")
            for t in range(NT):
                nc.tensor.matmul(prn[:, :], a2_bf[:, 2 * t:2 * t + 2],
                                 v_r[t][:, :],
                                 start=(t == 0), stop=(t == NT - 1))
            rr2 = rw.tile([2, 384], f32, tag="rr2")
            nc.scalar.copy(rr2[:, :], prn[:, :])
            r_row_f = rw.tile([1, 384], f32, tag="r_row_f")
            nc.sync.dma_start(r_row_f[0:1, :], rr2[1:2, :])
            r_row = rw.tile([1, 384], f32r, tag="r_row")
            nc.vector.tensor_scalar_mul(r_row[:, :], r_row_f[:, :], nkr[:, :])

            # ---------------- attention ----------------
            xoT = [bg.tile([128, NQ], f32r, tag=f"xT{j}", name=f"xoTt{j}") for j in range(3)]
            for hp in range(3):
                jq, jk = hp, 3 + hp
                xoh2 = [wr_p.tile([64, NQ], f32r, tag=f"xoh{par}", bufs=1,
                                  name=f"xoh{hp}_{par}") for par in range(2)]
                for ch in range(3):
                    Ec = {}
                    for t in range(NT):
                        for par in range(2):
                            pb = par * 64
                            sp = ps.tile([128, CH], f32, tag="mm",
                                         name=f"sp{hp}_{ch}_{t}_{par}")
                            nc.tensor.matmul(
                                sp[:, :],
                                qkT[jk][pb:pb + 64, t * 128:(t + 1) * 128],
                                qkT[jq][pb:pb + 64, ch * CH:(ch + 1) * CH],
                                start=True, stop=True, tile_position=(pb, 0))
                            e = ep.tile([128, CH], bf16, tag=f"Ec{t}_{par}",
                                        name=f"E{hp}_{ch}_{t}_{par}")
                            nc.scalar.activation(e[:, :], sp[:, :], ACT.Exp,
                                                 scale=0.125)
                            Ec[(t, par)] = e
                    for par in range(2):
                        h = 2 * hp + par
                        d2 = d2p.tile([2, CH], f32, tag="d2",
                                      name=f"d2_{hp}_{ch}_{par}")
                        for t in range(NT):
                            nc.tensor.matmul(d2[:, :], a2_bf[:, 2 * t:2 * t + 2],
                                             Ec[(t, par)][:, :],
                                             start=(t == 0), stop=(t == NT - 1))
                        w2 = wr_p.tile([2, CH], f32r, tag="w2",
                                       name=f"w2_{hp}_{ch}_{par}")
                        w2f = wr_p.tile([2, CH], f32, tag="w2f",
                                        name=f"w2f_{hp}_{ch}_{par}")
                        nc.vector.tensor_scalar_add(w2f[:, :], d2[:, :], nb2[:, :])
                        nc.vector.reciprocal(w2f[:, :], w2f[:, :])
                        nc.vector.tensor_mul(w2f[:, :], w2f[:, :],
                                             s2[:, ch * CH:(ch + 1) * CH])
                        nc.vector.tensor_copy(w2[:, :], w2f[:, :])
                        bc = bcp.tile([128, CH], f32, tag="bc",
                                      name=f"bc_{hp}_{ch}_{par}")
                        nc.tensor.matmul(bc[:, :], sID2[:, :], w2[:, :],
                                         start=True, stop=True)
                        wbc = wr_p.tile([128, CH], f32, tag="wbc",
                                        name=f"wbc_{hp}_{ch}_{par}")
                        nc.scalar.copy(wbc[:, :], bc[:, :])
                        pv = pvp.tile([128, CH], f32, tag="pv",
                                      name=f"pv_{hp}_{ch}_{par}")
                        for t in range(NT):
                            nc.tensor.matmul(pv[:, :],
                                             vt[t][:, 128 * h:128 * h + 128],
                                             Ec[(t, par)][:, :],
                                             start=(t == 0), stop=(t == NT - 1))
                        xow = wr_p.tile([128, CH], f32r, tag="xow",
                                        name=f"xow_{hp}_{ch}_{par}")
                        nc.vector.tensor_mul(xow[:, :], pv[:, :], wbc[:, :])
                        fin = pvp.tile([64, CH], f32, tag="pv",
                                       name=f"fin_{hp}_{ch}_{par}")
                        nc.tensor.matmul(fin[:, :], I2r[:, :], xow[:, :],
                                         start=True, stop=False)
                        nc.tensor.matmul(fin[:, :],
                                         r_row[0:1, 64 * h:64 * h + 64],
                                         s_rn_r[0:1, ch * CH:(ch + 1) * CH],
                                         start=False, stop=True)
                        nc.scalar.copy(xoh2[par][:, ch * CH:(ch + 1) * CH],
                                       fin[:, :])
                for par in range(2):
                    nc.sync.dma_start(xoT[hp][par * 64:(par + 1) * 64, :],
                                      xoh2[par][:, :])

            # ---------------- proj ----------------
            for mb in range(6):
                for ch in range(3):
                    pt = ps.tile([128, CH], f32, tag="mm")
                    for kc in range(3):
                        nc.tensor.matmul(pt[:, :],
                                         wp_r[kc][:, mb * 128:(mb + 1) * 128],
                                         xoT[kc][:, ch * CH:(ch + 1) * CH],
                                         start=(kc == 0), stop=(kc == 2))
                    ot = outp.tile([128, CH], f32, tag="ot")
                    nc.scalar.activation(ot[:, :], pt[:, :], ACT.Identity,
                                         bias=bp_t[:, mb:mb + 1])
                    w = min(CH, N - ch * CH)
                    nc.sync.dma_start(
                        out_d[mb * 128:(mb + 1) * 128, ch * CH:ch * CH + w],
                        ot[:, 0:w])
    nc.compile()
    return nc


def kernel(x, g_info, w_qkv, w_proj, b_proj):
    from concourse.bass_utils import run_bass_kernel_spmd

    if "nc" not in _cache:
        _cache["nc"] = _build()
    nc = _cache["nc"]

    x = np.ascontiguousarray(x, np.float32)
    g_info = np.ascontiguousarray(g_info, np.float32)
    w_qkv = np.ascontiguousarray(w_qkv, np.float32)
    w_proj = np.ascontiguousarray(w_proj, np.float32)
    b_proj = np.ascontiguousarray(b_proj, np.float32)

    g_rows = g_info[0, 0].reshape(H, Dh)
    bp_half = np.ascontiguousarray((b_proj / 2.0).reshape(6, 128).T)
    in_maps = []
    for c in range(NCORES):
        b, hh = c // 2, c % 2
        sel1 = np.zeros((4, 1), np.float32)
        sel1[b, 0] = 1.0
        wqk_c = np.concatenate(
            [w_qkv[:, hh * 384:(hh + 1) * 384],
             w_qkv[:, 768 + hh * 384:768 + (hh + 1) * 384]], axis=1)
        in_maps.append({
            "xb": x[b],
            "wqk": np.ascontiguousarray(wqk_c),
            "wv": np.ascontiguousarray(
                w_qkv[:, 1536 + hh * 384:1536 + (hh + 1) * 384]),
            "wp": np.ascontiguousarray(w_proj[hh * 384:(hh + 1) * 384, :]),
            "bp": bp_half,
            "g6": np.ascontiguousarray(g_rows[hh * 6:(hh + 1) * 6]),
            "sel": sel1,
        })
    res = run_bass_kernel_spmd(nc, in_maps, core_ids=list(range(NCORES)))
    _cache["last"] = res
    out = np.empty((B, N, C), np.float32)
    for b in range(B):
        acc = res.results[2 * b]["out"] + res.results[2 * b + 1]["out"]
        out[b] = acc.T
    return (out, g_info[1:])
